# revision 19
# baseline (speedup 1.0000x reference)
"""DGCNN point-cloud classifier on 8 Trainium2 NeuronCores.

Sharding: data-parallel over the batch dim B=8 - one point cloud per core.
Each core runs 4 dynamic-kNN edge-conv layers + the 512->1024 linear +
global max/mean pooling locally; the pooled [2048] vectors are AllGathered
and every core computes the (tiny) batch-norm MLP head redundantly.

Edge-conv algebra: h[p,k] = [x_p, x_j - x_p] @ W + b with max over k
  = (x_p @ (Wt - Wb) + b) + max_k (x_j @ Wb)
so only per-point features go through matmuls; the kNN gather moves rows of
c = x @ Wb with gpsimd ap_gather in a feature-major layout. Exact fp32
top-20 per row via DVE max8/match_replace/max_index.

The wrapped gather-index array (per gpsimd core group [16, 160], position
n = 128*k + p at (n%16, n//16), replicated into all 8 groups) is built with
ZERO DMAs: a masked broadcast-multiply followed by one tiny PE matmul
against 0/1 selection constants shipped from the host. k-major index order
makes the k-reduction a strided tensor_tensor max tree, which runs on the
otherwise-idle Pool engine. Layer 4 packs its 256 c-features as two fp16
halves in one u32 tensor so a single gather moves all of them.

Cross-layer overlap: each layer's prologue (R=2x, x^2, negsq, a/c matmuls)
is emitted per column-half as soon as the previous layer's first/second
four tiles finish, with AT/CT/NEGSQ double-buffered between layers; the
pooling matmuls similarly run in column-halves overlapped with layer 4.
"""
import numpy as np

N_CORES = 8
B, P, K, OUT = 8, 1024, 20, 40
T = P // 128  # 8 partition tiles per cloud
EPS = 1e-5
NEG = -1e30

# per-layer (C_in, F_out)
LAYERS = [(3, 64), (64, 64), (64, 128), (128, 256)]

_cache = {}


def _build():
    import concourse.bacc as bacc
    import concourse.mybir as mybir
    from concourse.tile import TileContext

    f32 = mybir.dt.float32
    f16 = mybir.dt.float16
    u16 = mybir.dt.uint16
    i16 = mybir.dt.int16
    u32 = mybir.dt.uint32
    f32r = mybir.dt.float32r

    nc = bacc.Bacc(None, num_devices=N_CORES)

    # ---------------- I/O ----------------
    posT = nc.dram_tensor("posT", [3, P], f32, kind="ExternalInput")
    wsub, whalf = [], []
    for li, (C, F) in enumerate(LAYERS):
        wsub.append(nc.dram_tensor(f"wsub{li}", [C, F], f32, kind="ExternalInput"))
        whalf.append(nc.dram_tensor(f"whalf{li}", [C, F], f32, kind="ExternalInput"))
    cst16 = nc.dram_tensor("cst16", [128, 288], f16, kind="ExternalInput")
    idt = nc.dram_tensor("idt", [128, 128], f32, kind="ExternalInput")
    misc = nc.dram_tensor("misc", [128, 32], f32, kind="ExternalInput")
    wm = nc.dram_tensor("wm", [128, 5120], f32, kind="ExternalInput")
    wa = nc.dram_tensor("wa", [128, 8192], f32, kind="ExternalInput")
    wbh = nc.dram_tensor("wbh", [128, 1024], f32, kind="ExternalInput")
    wc = nc.dram_tensor("wc", [128, 80], f32, kind="ExternalInput")
    y_out = nc.dram_tensor("y", [B, OUT], f32, kind="ExternalOutput")

    cc_in = nc.dram_tensor("cc_in", [1, 2048], f32, kind="Internal")
    cc_out = nc.dram_tensor("cc_out", [B, 2048], f32, kind="Internal",
                            addr_space="Shared")

    AG = mybir.AxisListType
    ALU = mybir.AluOpType
    ACTF = mybir.ActivationFunctionType

    with TileContext(nc) as tc:
        with tc.tile_pool(name="const", bufs=1) as cpool:
            # ---------------- resident SBUF tensors ----------------
            ONES = cpool.tile([1, P], f32)
            nc.vector.memset(ONES[:], 1.0)
            NEGCOL = cpool.tile([128, 1], f32)
            nc.vector.memset(NEGCOL[:], -1.0)
            EPSC = cpool.tile([128, 1], f32)
            nc.vector.memset(EPSC[:], EPS)

            # feature buffers (x^T per layer)
            L1 = cpool.tile([3, P], f32)
            R1 = cpool.tile([3, P], f32)
            L2 = cpool.tile([64, P], f32)
            R2 = cpool.tile([64, P], f32)
            L3 = cpool.tile([64, P], f32)
            R3 = cpool.tile([64, P], f32)
            L4 = cpool.tile([128, P], f32)
            R4 = cpool.tile([128, P], f32)
            Lbufs = [L1, L2, L3, L4]
            Rbufs = [R1, R2, R3, R4]
            # double-buffered across layers (prologue overlap)
            NEGa = cpool.tile([1, P], f32)
            NEGb = cpool.tile([1, P], f32)
            AT1a = cpool.tile([128, P], f32)
            AT1b = cpool.tile([128, P], f32)
            AT2 = cpool.tile([128, P], f32)
            CT1a = cpool.tile([128, P], f32)
            CT1b = cpool.tile([128, P], f32)
            CT4 = cpool.tile([128, P], u32)   # L4 packed (f16 lo=mt0, hi=mt1)
            X4a = cpool.tile([128, P], f32)
            X4b = cpool.tile([128, P], f32)
            # cat k-tiles rounded into f32r for the pooling matmul
            catr1 = cpool.tile([64, P], f32r)
            catr2 = cpool.tile([64, P], f32r)
            catr3 = cpool.tile([128, P], f32r)
            catr4a = cpool.tile([128, P], f32r)
            catr4b = cpool.tile([128, P], f32r)

            ws_sb, wh_sb = [], []
            for li, (C, F) in enumerate(LAYERS):
                w1 = cpool.tile([C, F], f32, tag=f"ws{li}")
                w2 = cpool.tile([C, F], f32, tag=f"wh{li}")
                ws_sb.append(w1)
                wh_sb.append(w2)
            EEMC = cpool.tile([128, 288], f16)
            IDT = cpool.tile([128, 128], f32)
            MISC = cpool.tile([128, 32], f32)
            nc.sync.dma_start(L1[0:3, :], posT[:])
            nc.sync.dma_start(ws_sb[0][:], wsub[0][:])
            nc.sync.dma_start(wh_sb[0][:], whalf[0][:])
            nc.sync.dma_start(EEMC[:], cst16[:])
            nc.sync.dma_start(MISC[:], misc[:])
            nc.sync.dma_start(IDT[:], idt[:])
            EE = EEMC[:, 0:128]
            MC = EEMC[:, 128:288]
            # misc cols: 0:5 layer biases, 5:13 bm, 13:17 ba, 17:21 ga,
            # 21:25 bea, 25:27 bbh, 27:29 gb, 29:31 beb, 31 bc
            BCOL = [0, 1, 2, 3]

            wm_sb = cpool.tile([128, 5120], f32r)
            wa_sb = cpool.tile([128, 8192], f32)
            wbh_sb = cpool.tile([128, 1024], f32)
            wc_sb = cpool.tile([128, 80], f32)

            def load_big_weights():
                for li in (1, 2, 3):
                    nc.sync.dma_start(ws_sb[li][:], wsub[li][:])
                    nc.sync.dma_start(wh_sb[li][:], whalf[li][:])
                nc.sync.dma_start(wm_sb[:], wm[:].bitcast(f32r))
                nc.sync.dma_start(wbh_sb[:], wbh[:])
                nc.sync.dma_start(wc_sb[:], wc[:])
                nc.sync.dma_start(wa_sb[:], wa[:])

            with tc.tile_pool(name="ps", bufs=2, space="PSUM") as pspool, \
                 tc.tile_pool(name="ps2", bufs=2, space="PSUM") as ps2pool, \
                 tc.tile_pool(name="ibc", bufs=2, space="PSUM") as ibcpool, \
                 tc.tile_pool(name="work", bufs=2) as wpool, \
                 tc.tile_pool(name="tree", bufs=1) as tpool, \
                 tc.tile_pool(name="one", bufs=2) as opool, \
                 tc.tile_pool(name="gathp", bufs=2) as gpool, \
                 tc.tile_pool(name="idxp", bufs=2) as ipool:

                def lpars(li):
                    C, F = LAYERS[li]
                    NEGSQ = [NEGa, NEGb][li % 2]
                    if li == 3:
                        ATs, CTx = [AT1b, AT2], CT4
                    else:
                        ATs = [[AT1a, AT1b][li % 2]]
                        CTx = [CT1a, CT1b][li % 2]
                    return C, F, NEGSQ, ATs, CTx

                sqx_t = {}

                def emit_prologue(li, half):
                    C, F, NEGSQ, ATs, CTx = lpars(li)
                    Lb, Rb = Lbufs[li], Rbufs[li]
                    c0, c1 = 512 * half, 512 * (half + 1)
                    if li not in sqx_t:
                        sqx_t[li] = opool.tile([128, P], f32, tag="sqx", name=f"sqx{li}")
                    sqx = sqx_t[li]
                    with tc.high_priority(offset=-60):
                        nc.scalar.activation(Rb[0:C, c0:c1], Lb[0:C, c0:c1],
                                             ACTF.Copy, scale=2.0)
                        nc.scalar.activation(sqx[0:C, c0:c1], Lb[0:C, c0:c1],
                                             ACTF.Square)
                        nps = ps2pool.tile([128, 512], f32, tag="pre")
                        nc.tensor.matmul(nps[0:1, :], NEGCOL[0:C, :],
                                         sqx[0:C, c0:c1], start=True, stop=True)
                        nc.scalar.activation(NEGSQ[0:1, c0:c1], nps[0:1, :],
                                             ACTF.Copy)
                        CT4h = CT4[:].bitcast(f16).rearrange(
                            "c (n h) -> c n h", h=2)
                        for mt in range(len(ATs)):
                            Fm = min(128, F - 128 * mt)
                            aps = ps2pool.tile([128, 512], f32, tag="pre")
                            cps = ps2pool.tile([128, 512], f32, tag="pre")
                            nc.tensor.matmul(
                                aps[0:Fm, :],
                                ws_sb[li][:, 128 * mt:128 * mt + Fm],
                                Lb[0:C, c0:c1], start=True, stop=True)
                            nc.tensor.matmul(
                                cps[0:Fm, :],
                                wh_sb[li][:, 128 * mt:128 * mt + Fm],
                                Lb[0:C, c0:c1], start=True, stop=True)
                            nc.scalar.activation(
                                ATs[mt][0:Fm, c0:c1], aps[0:Fm, :],
                                ACTF.Identity,
                                bias=MISC[0:Fm, BCOL[li] + mt:BCOL[li] + mt + 1])
                            if li == 3:
                                nc.scalar.activation(
                                    CT4h[0:Fm, c0:c1, mt], cps[0:Fm, :],
                                    ACTF.Copy)
                            else:
                                nc.scalar.activation(
                                    CTx[0:Fm, c0:c1], cps[0:Fm, :], ACTF.Copy)

                # pooling state + emitters
                GPH = cpool.tile([128, 16], f32)   # gmax: col 8*half+mt
                GMH = cpool.tile([128, 16], f32)   # gmean sums
                cat_kts = [(catr1, 0, 64), (catr2, 0, 64), (catr3, 0, 128),
                           (catr4a, 0, 128), (catr4b, 0, 128)]
                wm_kts = [(0, 64, 0), (0, 64, 1024), (0, 128, 2048),
                          (0, 128, 3072), (0, 128, 4096)]

                def emit_catr123():
                    with tc.high_priority(offset=-60):
                        nc.scalar.activation(catr1[:], L2[0:64, :], ACTF.Copy)
                        nc.scalar.activation(catr2[:], L3[0:64, :], ACTF.Copy)
                        nc.scalar.activation(catr3[:], L4[:], ACTF.Copy)

                def emit_pooling_half(half):
                    c0, c1 = 512 * half, 512 * (half + 1)
                    with tc.high_priority(offset=-60):
                        nc.scalar.activation(catr4a[:, c0:c1], X4a[:, c0:c1],
                                             ACTF.Copy)
                        nc.scalar.activation(catr4b[:, c0:c1], X4b[:, c0:c1],
                                             ACTF.Copy)
                        for mt in range(8):
                            mc0, mc1 = 128 * mt, 128 * (mt + 1)
                            ops = ps2pool.tile([128, 512], f32, tag="pre")
                            for kt, ((buf, r0, r1_), (wr0, wr1, wco)) in \
                                    enumerate(zip(cat_kts, wm_kts)):
                                nc.tensor.matmul(
                                    ops[:, :],
                                    wm_sb[wr0:wr1, wco + mc0:wco + mc1],
                                    buf[r0:r1_, c0:c1],
                                    start=(kt == 0), stop=(kt == 4))
                            gcol = 8 * half + mt
                            nc.vector.tensor_reduce(
                                GPH[:, gcol:gcol + 1], ops[:], axis=AG.X,
                                op=ALU.max)
                            osb = wpool.tile([128, P], f32, tag="scr",
                                             name="osb")
                            nc.scalar.activation(
                                osb[:, 0:512], ops[:], ACTF.Copy,
                                accum_out=GMH[:, gcol:gcol + 1])

                load_big_weights()
                emit_prologue(0, 0)
                emit_prologue(0, 1)

                for li, (C, F) in enumerate(LAYERS):
                    C, F, NEGSQ, ATs, CTx = lpars(li)
                    Lb, Rb = Lbufs[li], Rbufs[li]
                    outs_mt = [Lbufs[li + 1]] if li < 3 else [X4a, X4b]
                    lhs_kts = [(Lb, C), (ONES, 1)]
                    rhs_kts = [(Rb, C), (NEGSQ, 1)]
                    if li == 3:
                        emit_catr123()

                    pending = []

                    def flush_pending(li=li, F=F, ATs=ATs, outs_mt=outs_mt):
                        with tc.high_priority(offset=-60):
                            for (g_, tc0, tc1) in pending:
                                r1 = tpool.tile([128, 128], f32, tag="r1")
                                if li < 3:
                                    # single-instr k-max: reduce over the
                                    # outer k dim via a transposed view
                                    gv = g_[:].bitcast(f32).rearrange(
                                        "c (k p) -> c p k", k=K)
                                    nc.vector.tensor_reduce(
                                        r1[0:F, :], gv[0:F], axis=AG.X,
                                        op=ALU.max)
                                    nc.gpsimd.tensor_add(
                                        outs_mt[0][0:F, tc0:tc1], r1[0:F, :],
                                        ATs[0][0:F, tc0:tc1])
                                else:
                                    g2v = g_[:].bitcast(f16).rearrange(
                                        "c (k q) -> c k q", k=K)
                                    s10 = tpool.tile([128, 2560], f16,
                                                     tag="s10")
                                    s5 = tpool.tile([128, 1280], f16, tag="s5")
                                    s2 = tpool.tile([128, 512], f16, tag="s2")
                                    s1 = r1[:].bitcast(f16)
                                    s10v = s10[:].rearrange(
                                        "c (k q) -> c k q", k=10)
                                    s5v = s5[:].rearrange(
                                        "c (k q) -> c k q", k=5)
                                    s2v = s2[:].rearrange(
                                        "c (k q) -> c k q", k=2)
                                    nc.vector.tensor_tensor(
                                        s10v, g2v[:, 0:10], g2v[:, 10:20],
                                        op=ALU.max)
                                    nc.vector.tensor_tensor(
                                        s5v, s10v[:, 0:5], s10v[:, 5:10],
                                        op=ALU.max)
                                    nc.vector.tensor_tensor(
                                        s2v, s5v[:, 0:2], s5v[:, 2:4],
                                        op=ALU.max)
                                    nc.vector.tensor_tensor(
                                        s1, s2v[:, 0], s2v[:, 1], op=ALU.max)
                                    nc.vector.tensor_tensor(
                                        s1, s1, s5v[:, 4], op=ALU.max)
                                    s1v = s1.rearrange("c (p h) -> c p h", h=2)
                                    for mt in range(2):
                                        xf = tpool.tile([128, 128], f32,
                                                        tag=f"xf{mt}")
                                        nc.scalar.activation(
                                            xf[:], s1v[:, :, mt], ACTF.Copy)
                                        nc.gpsimd.tensor_add(
                                            outs_mt[mt][:, tc0:tc1], xf[:],
                                            ATs[mt][:, tc0:tc1])
                        pending.clear()

                    for t in range(T):
                        tc0, tc1 = 128 * t, 128 * (t + 1)
                        sps = pspool.tile([128, P], f32, tag="s")
                        for n in range(2):
                            for kt, ((lb, kk), (rb, _)) in enumerate(
                                    zip(lhs_kts, rhs_kts)):
                                nc.tensor.matmul(
                                    sps[:, 512 * n:512 * (n + 1)],
                                    lb[0:kk, tc0:tc1],
                                    rb[0:kk, 512 * n:512 * (n + 1)],
                                    start=(kt == 0), stop=(kt == 1))

                        # exact fp32 top-20 (values + indices) per row
                        vv = ipool.tile([128, 24], f32, tag="vv")
                        idxc = ipool.tile([128, 24], u16, tag="idxc")
                        scr = wpool.tile([128, P], f32, tag="scr")
                        nc.vector.max(vv[:, 0:8], sps[:])
                        nc.vector.max_index(idxc[:, 0:8], vv[:, 0:8], sps[:])
                        nc.vector.match_replace(scr[:], vv[:, 0:8], sps[:], NEG)
                        nc.vector.max(vv[:, 8:16], scr[:])
                        nc.vector.max_index(idxc[:, 8:16], vv[:, 8:16], scr[:])
                        nc.vector.match_replace(scr[:], vv[:, 8:16], scr[:], NEG)
                        nc.vector.max(vv[:, 16:24], scr[:])
                        nc.vector.max_index(idxc[:, 16:24], vv[:, 16:24], scr[:])

                        # wrapped idx array via maskmul + PE matmul (no DMAs):
                        # idxw[16g+r, 8k+q] = idxc[16q+r, k]  (n = 128k+p)
                        idxf = ipool.tile([128, K], f16, tag="idxf")
                        nc.scalar.activation(idxf[:], idxc[:, 0:K], ACTF.Copy)
                        rhs = ipool.tile([128, 8 * K], f16, tag="rhs")
                        nc.gpsimd.tensor_tensor(
                            rhs[:].rearrange("p (k g) -> p k g", g=8),
                            idxf[:].unsqueeze(-1).broadcast_to([128, K, 8]),
                            MC.rearrange("p (k g) -> p k g", g=8),
                            op=ALU.mult)
                        ibc = ibcpool.tile([128, 8 * K], f32, tag="ibc")
                        nc.tensor.matmul(ibc[:], EE, rhs[:], start=True, stop=True)
                        idxw = ipool.tile([128, 8 * K], i16, tag="idxw")
                        nc.scalar.activation(idxw[:].bitcast(u16), ibc[:], ACTF.Copy)

                        flush_pending()
                        if t == 4:
                            if li < 3:
                                emit_prologue(li + 1, 0)
                            else:
                                emit_pooling_half(0)

                        # gather c rows (k-major output: [F, K, 128])
                        gath = gpool.tile([128, K * 128], u32, tag="gath")
                        if li < 3:
                            Fg = ((F + 15) // 16) * 16
                            nc.gpsimd.ap_gather(
                                gath[0:Fg, :].bitcast(f32),
                                CTx[0:Fg, :].rearrange("c (n d) -> c n d", d=1),
                                idxw[0:Fg, :],
                                channels=Fg, num_elems=P, d=1, num_idxs=K * 128)
                        else:
                            nc.gpsimd.ap_gather(
                                gath[:],
                                CTx[:].rearrange("c (n d) -> c n d", d=1),
                                idxw[:],
                                channels=128, num_elems=P, d=1, num_idxs=K * 128)
                        pending.append((gath, tc0, tc1))

                    flush_pending()
                    if li < 3:
                        emit_prologue(li + 1, 1)
                    else:
                        emit_pooling_half(1)

                # ---------------- finish pooling ----------------
                GP = cpool.tile([128, 16], f32)
                nc.vector.tensor_tensor(GP[:, 0:8], GPH[:, 0:8], GPH[:, 8:16],
                                        op=ALU.max)
                nc.vector.tensor_tensor(GP[:, 0:8], GP[:, 0:8], MISC[:, 5:13],
                                        op=ALU.add)
                nc.vector.tensor_tensor(GP[:, 8:16], GMH[:, 0:8], GMH[:, 8:16],
                                        op=ALU.add)
                nc.vector.tensor_scalar(GP[:, 8:16], GP[:, 8:16], 1.0 / P, None,
                                        op0=ALU.mult)
                nc.vector.tensor_tensor(GP[:, 8:16], GP[:, 8:16], MISC[:, 5:13],
                                        op=ALU.add)

                # pooled [2048] -> cc_in via PE transpose (one DMA), AllGather
                gpt_ps = ibcpool.tile([128, 8 * K], f32, tag="ibc")
                nc.tensor.transpose(gpt_ps[0:16, 0:128], GP[:], IDT[:])
                GPT = wpool.tile([16, 128], f32, tag="gpt")
                nc.scalar.activation(GPT[:], gpt_ps[0:16, 0:128], ACTF.Copy)
                nc.sync.dma_start(
                    cc_in[:].rearrange("a (m f) -> m (a f)", m=16), GPT[:])
                nc.gpsimd.collective_compute(
                    "AllGather", ALU.bypass,
                    replica_groups=[list(range(N_CORES))],
                    ins=[cc_in[:].opt()], outs=[cc_out[:].opt()])

                # ---------------- head (redundant on every core) ----------------
                HTraw = gpool.tile([128, K * 128], u32, tag="gath")
                HTrawv = HTraw[0:8, 0:2048].bitcast(f32)
                nc.sync.dma_start(HTrawv, cc_out[:])
                ht_ps = ibcpool.tile([128, 8 * K], f32, tag="ibc")
                for k in range(16):
                    nc.tensor.transpose(ht_ps[:, 8 * k:8 * (k + 1)],
                                        HTrawv[:, 128 * k:128 * (k + 1)],
                                        IDT[0:8, 0:8])
                HT = cpool.tile([128, 128], f32)
                nc.scalar.activation(HT[:], ht_ps[:, 0:128], ACTF.Copy)

                def bn_leaky(src, blocks, gcol):
                    # src [128, 8*blocks]; batch-norm over batch then leaky,
                    # vectorized across blocks. gamma at MISC col gcol..,
                    # beta at gcol+blocks..
                    sv = src.rearrange("c (b e) -> c b e", b=blocks)
                    mu = wpool.tile([128, 4], f32, tag="mu")
                    nc.vector.tensor_reduce(mu[:, 0:blocks], sv, axis=AG.X,
                                            op=ALU.add)
                    nc.vector.tensor_scalar(mu[:, 0:blocks], mu[:, 0:blocks],
                                            1.0 / 8, None, op0=ALU.mult)
                    nc.vector.tensor_tensor(
                        sv, sv,
                        mu[:, 0:blocks].unsqueeze(-1).broadcast_to(
                            [128, blocks, 8]), op=ALU.subtract)
                    sq2 = wpool.tile([128, 32], f32, tag="sq2")
                    nc.scalar.activation(sq2[:, 0:8 * blocks], src, ACTF.Square)
                    var = wpool.tile([128, 4], f32, tag="var")
                    nc.vector.tensor_reduce(
                        var[:, 0:blocks],
                        sq2[:, 0:8 * blocks].rearrange("c (b e) -> c b e",
                                                       b=blocks),
                        axis=AG.X, op=ALU.add)
                    nc.scalar.activation(var[:, 0:blocks], var[:, 0:blocks],
                                         ACTF.Sqrt, scale=1.0 / 8, bias=EPSC[:])
                    nc.vector.reciprocal(var[:, 0:blocks], var[:, 0:blocks])
                    nc.vector.tensor_tensor(var[:, 0:blocks], var[:, 0:blocks],
                                            MISC[:, gcol:gcol + blocks],
                                            op=ALU.mult)
                    nc.vector.tensor_tensor(
                        sv, sv,
                        var[:, 0:blocks].unsqueeze(-1).broadcast_to(
                            [128, blocks, 8]), op=ALU.mult)
                    nc.vector.tensor_tensor(
                        sv, sv,
                        MISC[:, gcol + blocks:gcol + 2 * blocks].unsqueeze(
                            -1).broadcast_to([128, blocks, 8]), op=ALU.add)
                    lk = wpool.tile([128, 32], f32, tag="lk")
                    nc.vector.tensor_scalar(lk[:, 0:8 * blocks], src, 0.2,
                                            None, op0=ALU.mult)
                    nc.vector.tensor_tensor(src, src, lk[:, 0:8 * blocks],
                                            op=ALU.max)

                HA = cpool.tile([128, 32], f32)
                for mt in range(4):
                    hps = ibcpool.tile([128, 8 * K], f32, tag="ibc")
                    for k in range(16):
                        nc.tensor.matmul(
                            hps[:, 0:8],
                            wa_sb[:, 512 * k + 128 * mt:512 * k + 128 * (mt + 1)],
                            HT[:, 8 * k:8 * (k + 1)],
                            start=(k == 0), stop=(k == 15))
                    nc.scalar.activation(HA[:, 8 * mt:8 * (mt + 1)], hps[:, 0:8],
                                         ACTF.Identity,
                                         bias=MISC[:, 13 + mt:14 + mt])
                bn_leaky(HA[:], 4, 17)

                HB = cpool.tile([128, 16], f32)
                for mt in range(2):
                    hps = ibcpool.tile([128, 8 * K], f32, tag="ibc")
                    for k in range(4):
                        nc.tensor.matmul(
                            hps[:, 0:8],
                            wbh_sb[:, 256 * k + 128 * mt:256 * k + 128 * (mt + 1)],
                            HA[:, 8 * k:8 * (k + 1)],
                            start=(k == 0), stop=(k == 3))
                    nc.scalar.activation(HB[:, 8 * mt:8 * (mt + 1)], hps[:, 0:8],
                                         ACTF.Identity,
                                         bias=MISC[:, 25 + mt:26 + mt])
                bn_leaky(HB[:], 2, 27)

                ops2 = ibcpool.tile([128, 8 * K], f32, tag="ibc")
                for k in range(2):
                    nc.tensor.matmul(ops2[0:40, 0:8],
                                     wc_sb[:, 40 * k:40 * (k + 1)],
                                     HB[:, 8 * k:8 * (k + 1)],
                                     start=(k == 0), stop=(k == 1))
                outs = cpool.tile([40, 8], f32)
                nc.scalar.activation(outs[:], ops2[0:40, 0:8], ACTF.Identity,
                                     bias=MISC[0:40, 31:32])
                nc.sync.dma_start(y_out[:].rearrange("b f -> f b"), outs[:])

    nc.finalize()
    return nc


def _prep_inputs(inputs):
    """Host-side sharding + weight reparametrization; all fp32."""
    f = np.float32
    pos = np.ascontiguousarray(inputs["pos"], dtype=f).reshape(B, P, 3)
    names = [("W1", "b1"), ("W2", "b2"), ("W3", "b3"), ("W4", "b4")]
    common = {}
    mis = np.zeros((128, 32), dtype=f)
    for li, (C, F) in enumerate(LAYERS):
        W = np.asarray(inputs[names[li][0]], dtype=f)
        b = np.asarray(inputs[names[li][1]], dtype=f)
        common[f"wsub{li}"] = np.ascontiguousarray(W[:C] - W[C:])
        common[f"whalf{li}"] = np.ascontiguousarray(W[C:])
        if li < 3:
            mis[0:F, li] = b
        else:
            mis[:, 3] = b[0:128]
            mis[:, 4] = b[128:256]
    mis[:, 5:13] = np.asarray(inputs["bm"], dtype=f).reshape(8, 128).T
    mis[:, 13:17] = np.asarray(inputs["ba"], dtype=f).reshape(4, 128).T
    mis[:, 17:21] = np.asarray(inputs["ga"], dtype=f).reshape(4, 128).T
    mis[:, 21:25] = np.asarray(inputs["bea"], dtype=f).reshape(4, 128).T
    mis[:, 25:27] = np.asarray(inputs["bb"], dtype=f).reshape(2, 128).T
    mis[:, 27:29] = np.asarray(inputs["gb"], dtype=f).reshape(2, 128).T
    mis[:, 29:31] = np.asarray(inputs["beb"], dtype=f).reshape(2, 128).T
    mis[0:40, 31] = np.asarray(inputs["bc"], dtype=f)
    common["misc"] = mis
    # selection consts for the idx-wrap transform
    p = np.arange(128)
    i = np.arange(128)
    EE = (p[:, None] % 16 == i[None, :] % 16).astype(np.float16)
    g = np.arange(8)
    MCm = np.broadcast_to(
        (p[:, None, None] // 16 == g[None, None, :]), (128, K, 8))
    MC = MCm.astype(np.float16).reshape(128, 8 * K)
    common["cst16"] = np.ascontiguousarray(np.concatenate([EE, MC], axis=1))
    common["idt"] = np.eye(128, dtype=f)
    # wm pack [128, 5120]: k-tile i in col block 1024*i, rows 0:ki
    Wm = np.asarray(inputs["Wm"], dtype=f)
    wmp = np.zeros((128, 5120), dtype=f)
    wmp[0:64, 0:1024] = Wm[0:64]
    wmp[0:64, 1024:2048] = Wm[64:128]
    wmp[:, 2048:3072] = Wm[128:256]
    wmp[:, 3072:4096] = Wm[256:384]
    wmp[:, 4096:5120] = Wm[384:512]
    common["wm"] = wmp
    Wa = np.asarray(inputs["Wa"], dtype=f)  # [2048, 512]
    common["wa"] = np.ascontiguousarray(
        Wa.reshape(16, 128, 512).transpose(1, 0, 2).reshape(128, 8192))
    Wb = np.asarray(inputs["Wb"], dtype=f)  # [512, 256]
    common["wbh"] = np.ascontiguousarray(
        Wb.reshape(4, 128, 256).transpose(1, 0, 2).reshape(128, 1024))
    Wc = np.asarray(inputs["Wc"], dtype=f)  # [256, 40]
    common["wc"] = np.ascontiguousarray(
        Wc.reshape(2, 128, 40).transpose(1, 0, 2).reshape(128, 80))
    maps = []
    for c in range(N_CORES):
        m = dict(common)
        m["posT"] = np.ascontiguousarray(pos[c].T)
        maps.append(m)
    return maps


def kernel(**inputs) -> np.ndarray:
    from concourse.bass_utils import run_bass_kernel_spmd

    if "nc" not in _cache:
        _cache["nc"] = _build()
    nc = _cache["nc"]
    in_maps = _prep_inputs(inputs)
    res = run_bass_kernel_spmd(nc, in_maps, core_ids=list(range(N_CORES)))
    return np.asarray(res.results[0]["y"], dtype=np.float32)


# revision 25
# speedup vs baseline: 1.0318x; 1.0318x over previous
"""DGCNN point-cloud classifier on 8 Trainium2 NeuronCores.

Sharding: data-parallel over the batch dim B=8 - one point cloud per core.
Each core runs 4 dynamic-kNN edge-conv layers + the 512->1024 linear +
global max/mean pooling locally; the pooled [2048] vectors are AllGathered
and every core computes the (tiny) batch-norm MLP head redundantly.

Edge-conv algebra: h[p,k] = [x_p, x_j - x_p] @ W + b with max over k
  = (x_p @ (Wt - Wb) + b) + max_k (x_j @ Wb)
so only per-point features go through matmuls; the kNN gather moves rows of
c = x @ Wb with gpsimd ap_gather in a feature-major layout. Exact fp32
top-20 per row via DVE max8/match_replace/max_index.

The wrapped gather-index array (per gpsimd core group [16, 160], position
n = 128*k + p at (n%16, n//16), replicated into all 8 groups) is built with
ZERO DMAs: a masked broadcast-multiply followed by one tiny PE matmul
against 0/1 selection constants shipped from the host. k-major index order
makes the k-reduction a strided tensor_tensor max tree, which runs on the
otherwise-idle Pool engine. Layer 4 packs its 256 c-features as two fp16
halves in one u32 tensor so a single gather moves all of them.

Cross-layer overlap: each layer's prologue (R=2x, x^2, negsq, a/c matmuls)
is emitted per column-half as soon as the previous layer's first/second
four tiles finish, with AT/CT/NEGSQ double-buffered between layers; the
pooling matmuls similarly run in column-halves overlapped with layer 4.
"""
import numpy as np

N_CORES = 8
B, P, K, OUT = 8, 1024, 20, 40
T = P // 128  # 8 partition tiles per cloud
EPS = 1e-5
NEG = -1e30

# per-layer (C_in, F_out)
LAYERS = [(3, 64), (64, 64), (64, 128), (128, 256)]

_cache = {}


def _build():
    import concourse.bacc as bacc
    import concourse.mybir as mybir
    from concourse.tile import TileContext

    f32 = mybir.dt.float32
    f16 = mybir.dt.float16
    u16 = mybir.dt.uint16
    i16 = mybir.dt.int16
    u32 = mybir.dt.uint32
    f32r = mybir.dt.float32r

    nc = bacc.Bacc(None, num_devices=N_CORES)

    # ---------------- I/O ----------------
    posT = nc.dram_tensor("posT", [3, P], f32, kind="ExternalInput")
    wsub, whalf = [], []
    for li, (C, F) in enumerate(LAYERS):
        wsub.append(nc.dram_tensor(f"wsub{li}", [C, F], f32, kind="ExternalInput"))
        whalf.append(nc.dram_tensor(f"whalf{li}", [C, F], f32, kind="ExternalInput"))
    cst16 = nc.dram_tensor("cst16", [128, 288], f16, kind="ExternalInput")
    idt = nc.dram_tensor("idt", [128, 128], f32, kind="ExternalInput")
    misc = nc.dram_tensor("misc", [128, 32], f32, kind="ExternalInput")
    wm = nc.dram_tensor("wm", [128, 5120], f32, kind="ExternalInput")
    wa = nc.dram_tensor("wa", [128, 8192], f32, kind="ExternalInput")
    wbh = nc.dram_tensor("wbh", [128, 1024], f32, kind="ExternalInput")
    wc = nc.dram_tensor("wc", [128, 80], f32, kind="ExternalInput")
    y_out = nc.dram_tensor("y", [B, OUT], f32, kind="ExternalOutput")

    cc_in = nc.dram_tensor("cc_in", [1, 2048], f32, kind="Internal")
    cc_out = nc.dram_tensor("cc_out", [B, 2048], f32, kind="Internal",
                            addr_space="Shared")

    AG = mybir.AxisListType
    ALU = mybir.AluOpType
    ACTF = mybir.ActivationFunctionType

    with TileContext(nc) as tc:
        with tc.tile_pool(name="const", bufs=1) as cpool:
            # ---------------- resident SBUF tensors ----------------
            ONES = cpool.tile([1, P], f32)
            nc.vector.memset(ONES[:], 1.0)
            NEGCOL = cpool.tile([128, 1], f32)
            nc.vector.memset(NEGCOL[:], -1.0)
            EPSC = cpool.tile([128, 1], f32)
            nc.vector.memset(EPSC[:], EPS)

            # feature buffers (x^T per layer)
            L1 = cpool.tile([4, P], f32)
            R1 = cpool.tile([4, P], f32)
            L2 = cpool.tile([65, P], f32)
            R2 = cpool.tile([65, P], f32)
            L3 = cpool.tile([65, P], f32)
            R3 = cpool.tile([65, P], f32)
            L4 = cpool.tile([128, P], f32)
            R4 = cpool.tile([128, P], f32)
            Lbufs = [L1, L2, L3, L4]
            Rbufs = [R1, R2, R3, R4]
            # double-buffered across layers (prologue overlap)
            NEGa = cpool.tile([1, P], f32)
            NEGb = cpool.tile([1, P], f32)
            AT1a = cpool.tile([128, P], f32)
            AT1b = cpool.tile([128, P], f32)
            AT2 = cpool.tile([128, P], f32)
            CT1a = cpool.tile([128, P], f32)
            CT1b = cpool.tile([128, P], f32)
            CT4 = cpool.tile([128, P], u32)   # L4 packed (f16 lo=mt0, hi=mt1)
            X4a = cpool.tile([128, P], f32)
            X4b = cpool.tile([128, P], f32)
            # cat k-tiles rounded into f32r for the pooling matmul
            catr1 = cpool.tile([64, P], f32r)
            catr2 = cpool.tile([64, P], f32r)
            catr3 = cpool.tile([128, P], f32r)
            catr4a = cpool.tile([128, P], f32r)
            catr4b = cpool.tile([128, P], f32r)

            ws_sb, wh_sb = [], []
            for li, (C, F) in enumerate(LAYERS):
                w1 = cpool.tile([C, F], f32, tag=f"ws{li}")
                w2 = cpool.tile([C, F], f32, tag=f"wh{li}")
                ws_sb.append(w1)
                wh_sb.append(w2)
            EEMC = cpool.tile([128, 288], f16)
            IDT = cpool.tile([128, 128], f32)
            MISC = cpool.tile([128, 32], f32)
            nc.sync.dma_start(L1[0:3, :], posT[:])
            nc.sync.dma_start(ws_sb[0][:], wsub[0][:])
            nc.sync.dma_start(wh_sb[0][:], whalf[0][:])
            nc.sync.dma_start(EEMC[:], cst16[:])
            nc.sync.dma_start(MISC[:], misc[:])
            nc.sync.dma_start(IDT[:], idt[:])
            nc.sync.dma_start(L1[3:4, :], ONES[:])
            nc.sync.dma_start(L2[64:65, :], ONES[:])
            nc.sync.dma_start(L3[64:65, :], ONES[:])
            EE = EEMC[:, 0:128]
            MC = EEMC[:, 128:288]
            # misc cols: 0:5 layer biases, 5:13 bm, 13:17 ba, 17:21 ga,
            # 21:25 bea, 25:27 bbh, 27:29 gb, 29:31 beb, 31 bc
            BCOL = [0, 1, 2, 3]

            wm_sb = cpool.tile([128, 5120], f32r)
            wa_sb = cpool.tile([128, 8192], f32)
            wbh_sb = cpool.tile([128, 1024], f32)
            wc_sb = cpool.tile([128, 80], f32)

            def load_big_weights():
                for li in (1, 2, 3):
                    nc.sync.dma_start(ws_sb[li][:], wsub[li][:])
                    nc.sync.dma_start(wh_sb[li][:], whalf[li][:])
                nc.sync.dma_start(wm_sb[:], wm[:].bitcast(f32r))
                nc.sync.dma_start(wbh_sb[:], wbh[:])
                nc.sync.dma_start(wc_sb[:], wc[:])
                nc.sync.dma_start(wa_sb[:], wa[:])

            with tc.tile_pool(name="ps", bufs=2, space="PSUM") as pspool, \
                 tc.tile_pool(name="ps2", bufs=2, space="PSUM") as ps2pool, \
                 tc.tile_pool(name="ibc", bufs=2, space="PSUM") as ibcpool, \
                 tc.tile_pool(name="work", bufs=2) as wpool, \
                 tc.tile_pool(name="tree", bufs=1) as tpool, \
                 tc.tile_pool(name="one", bufs=2) as opool, \
                 tc.tile_pool(name="gathp", bufs=2) as gpool, \
                 tc.tile_pool(name="idxp", bufs=2) as ipool:

                def lpars(li):
                    C, F = LAYERS[li]
                    NEGSQ = [NEGa, NEGb][li % 2]
                    if li == 3:
                        ATs, CTx = [AT1b, AT2], CT4
                    else:
                        ATs = [[AT1a, AT1b][li % 2]]
                        CTx = [CT1a, CT1b][li % 2]
                    return C, F, NEGSQ, ATs, CTx

                sqx_t = {}

                def emit_prologue(li, half):
                    C, F, NEGSQ, ATs, CTx = lpars(li)
                    Lb, Rb = Lbufs[li], Rbufs[li]
                    c0, c1 = 512 * half, 512 * (half + 1)
                    if li not in sqx_t:
                        sqx_t[li] = opool.tile([128, P], f32, tag="sqx", name=f"sqx{li}")
                    sqx = sqx_t[li]
                    with tc.high_priority(offset=-60):
                        nc.scalar.activation(Rb[0:C, c0:c1], Lb[0:C, c0:c1],
                                             ACTF.Copy, scale=2.0)
                        nc.scalar.activation(sqx[0:C, c0:c1], Lb[0:C, c0:c1],
                                             ACTF.Square)
                        nps = ps2pool.tile([128, 512], f32, tag="pre")
                        nc.tensor.matmul(nps[0:1, :], NEGCOL[0:C, :],
                                         sqx[0:C, c0:c1], start=True, stop=True)
                        nc.scalar.activation(NEGSQ[0:1, c0:c1], nps[0:1, :],
                                             ACTF.Copy)
                        CT4h = CT4[:].bitcast(f16).rearrange(
                            "c (n h) -> c n h", h=2)
                        for mt in range(len(ATs)):
                            Fm = min(128, F - 128 * mt)
                            aps = ps2pool.tile([128, 512], f32, tag="pre")
                            cps = ps2pool.tile([128, 512], f32, tag="pre")
                            nc.tensor.matmul(
                                aps[0:Fm, :],
                                ws_sb[li][:, 128 * mt:128 * mt + Fm],
                                Lb[0:C, c0:c1], start=True, stop=True)
                            nc.tensor.matmul(
                                cps[0:Fm, :],
                                wh_sb[li][:, 128 * mt:128 * mt + Fm],
                                Lb[0:C, c0:c1], start=True, stop=True)
                            nc.scalar.activation(
                                ATs[mt][0:Fm, c0:c1], aps[0:Fm, :],
                                ACTF.Identity,
                                bias=MISC[0:Fm, BCOL[li] + mt:BCOL[li] + mt + 1])
                            if li == 3:
                                nc.scalar.activation(
                                    CT4h[0:Fm, c0:c1, mt], cps[0:Fm, :],
                                    ACTF.Copy)
                            else:
                                nc.scalar.activation(
                                    CTx[0:Fm, c0:c1], cps[0:Fm, :], ACTF.Copy)

                # pooling state + emitters
                GPH = cpool.tile([128, 16], f32)   # gmax: col 8*half+mt
                GMH = cpool.tile([128, 16], f32)   # gmean sums
                cat_kts = [(catr1, 0, 64), (catr2, 0, 64), (catr3, 0, 128),
                           (catr4a, 0, 128), (catr4b, 0, 128)]
                wm_kts = [(0, 64, 0), (0, 64, 1024), (0, 128, 2048),
                          (0, 128, 3072), (0, 128, 4096)]

                def emit_catr123():
                    with tc.high_priority(offset=-60):
                        nc.scalar.activation(catr1[:], L2[0:64, :], ACTF.Copy)
                        nc.scalar.activation(catr2[:], L3[0:64, :], ACTF.Copy)
                        nc.scalar.activation(catr3[:], L4[:], ACTF.Copy)

                def emit_pooling_half(half):
                    c0, c1 = 512 * half, 512 * (half + 1)
                    with tc.high_priority(offset=-60):
                        nc.scalar.activation(catr4a[:, c0:c1], X4a[:, c0:c1],
                                             ACTF.Copy)
                        nc.scalar.activation(catr4b[:, c0:c1], X4b[:, c0:c1],
                                             ACTF.Copy)
                        for mt in range(8):
                            mc0, mc1 = 128 * mt, 128 * (mt + 1)
                            ops = ps2pool.tile([128, 512], f32, tag="pre")
                            for kt, ((buf, r0, r1_), (wr0, wr1, wco)) in \
                                    enumerate(zip(cat_kts, wm_kts)):
                                nc.tensor.matmul(
                                    ops[:, :],
                                    wm_sb[wr0:wr1, wco + mc0:wco + mc1],
                                    buf[r0:r1_, c0:c1],
                                    start=(kt == 0), stop=(kt == 4))
                            gcol = 8 * half + mt
                            nc.vector.tensor_reduce(
                                GPH[:, gcol:gcol + 1], ops[:], axis=AG.X,
                                op=ALU.max)
                            osb = wpool.tile([128, P], f32, tag="scr",
                                             name="osb")
                            nc.scalar.activation(
                                osb[:, 0:512], ops[:], ACTF.Copy,
                                accum_out=GMH[:, gcol:gcol + 1])

                load_big_weights()
                emit_prologue(0, 0)
                emit_prologue(0, 1)

                for li, (C, F) in enumerate(LAYERS):
                    C, F, NEGSQ, ATs, CTx = lpars(li)
                    Lb, Rb = Lbufs[li], Rbufs[li]
                    outs_mt = [Lbufs[li + 1]] if li < 3 else [X4a, X4b]
                    lhs_kts = [(Lb, C), (ONES, 1)]
                    rhs_kts = [(Rb, C), (NEGSQ, 1)]
                    if li == 3:
                        emit_catr123()

                    pending = []

                    def flush_pending(li=li, F=F, ATs=ATs, outs_mt=outs_mt):
                        with tc.high_priority(offset=-60):
                            for (g_, tc0, tc1) in pending:
                                r1 = tpool.tile([128, 128], f32, tag="r1")
                                if li < 3:
                                    # single-instr k-max: reduce over the
                                    # outer k dim via a transposed view
                                    gv = g_[:].bitcast(f32).rearrange(
                                        "c (k p) -> c p k", k=K)
                                    nc.vector.tensor_reduce(
                                        r1[0:F, :], gv[0:F], axis=AG.X,
                                        op=ALU.max)
                                    nc.gpsimd.tensor_add(
                                        outs_mt[0][0:F, tc0:tc1], r1[0:F, :],
                                        ATs[0][0:F, tc0:tc1])
                                else:
                                    g2v = g_[:].bitcast(f16).rearrange(
                                        "c (k q) -> c k q", k=K)
                                    s10 = tpool.tile([128, 2560], f16,
                                                     tag="s10")
                                    s5 = tpool.tile([128, 1280], f16, tag="s5")
                                    s2 = tpool.tile([128, 512], f16, tag="s2")
                                    s1 = r1[:].bitcast(f16)
                                    s10v = s10[:].rearrange(
                                        "c (k q) -> c k q", k=10)
                                    s5v = s5[:].rearrange(
                                        "c (k q) -> c k q", k=5)
                                    s2v = s2[:].rearrange(
                                        "c (k q) -> c k q", k=2)
                                    nc.vector.tensor_tensor(
                                        s10v, g2v[:, 0:10], g2v[:, 10:20],
                                        op=ALU.max)
                                    nc.vector.tensor_tensor(
                                        s5v, s10v[:, 0:5], s10v[:, 5:10],
                                        op=ALU.max)
                                    nc.vector.tensor_tensor(
                                        s2v, s5v[:, 0:2], s5v[:, 2:4],
                                        op=ALU.max)
                                    nc.vector.tensor_tensor(
                                        s1, s2v[:, 0], s2v[:, 1], op=ALU.max)
                                    nc.vector.tensor_tensor(
                                        s1, s1, s5v[:, 4], op=ALU.max)
                                    s1v = s1.rearrange("c (p h) -> c p h", h=2)
                                    for mt in range(2):
                                        xf = tpool.tile([128, 128], f32,
                                                        tag=f"xf{mt}")
                                        nc.scalar.activation(
                                            xf[:], s1v[:, :, mt], ACTF.Copy)
                                        nc.gpsimd.tensor_add(
                                            outs_mt[mt][:, tc0:tc1], xf[:],
                                            ATs[mt][:, tc0:tc1])
                        pending.clear()

                    for t in range(T):
                        tc0, tc1 = 128 * t, 128 * (t + 1)
                        sps = pspool.tile([128, P], f32, tag="s")
                        for n in range(2):
                            for kt, ((lb, kk), (rb, _)) in enumerate(
                                    zip(lhs_kts, rhs_kts)):
                                nc.tensor.matmul(
                                    sps[:, 512 * n:512 * (n + 1)],
                                    lb[0:kk, tc0:tc1],
                                    rb[0:kk, 512 * n:512 * (n + 1)],
                                    start=(kt == 0),
                                    stop=(kt == len(lhs_kts) - 1))

                        # exact fp32 top-20 (values + indices) per row
                        vv = ipool.tile([128, 24], f32, tag="vv")
                        idxc = ipool.tile([128, 24], u16, tag="idxc")
                        scr = wpool.tile([128, P], f32, tag="scr")
                        nc.vector.max(vv[:, 0:8], sps[:])
                        nc.vector.max_index(idxc[:, 0:8], vv[:, 0:8], sps[:])
                        nc.vector.match_replace(scr[:], vv[:, 0:8], sps[:], NEG)
                        nc.vector.max(vv[:, 8:16], scr[:])
                        nc.vector.max_index(idxc[:, 8:16], vv[:, 8:16], scr[:])
                        nc.vector.match_replace(scr[:], vv[:, 8:16], scr[:], NEG)
                        nc.vector.max(vv[:, 16:24], scr[:])
                        nc.vector.max_index(idxc[:, 16:24], vv[:, 16:24], scr[:])

                        # wrapped idx array via maskmul + PE matmul (no DMAs):
                        # idxw[16g+r, 8k+q] = idxc[16q+r, k]  (n = 128k+p)
                        idxf = ipool.tile([128, K], f16, tag="idxf")
                        nc.scalar.activation(idxf[:], idxc[:, 0:K], ACTF.Copy)
                        rhs = ipool.tile([128, 8 * K], f16, tag="rhs")
                        nc.gpsimd.tensor_tensor(
                            rhs[:].rearrange("p (k g) -> p k g", g=8),
                            idxf[:].unsqueeze(-1).broadcast_to([128, K, 8]),
                            MC.rearrange("p (k g) -> p k g", g=8),
                            op=ALU.mult)
                        ibc = ibcpool.tile([128, 8 * K], f32, tag="ibc")
                        nc.tensor.matmul(ibc[:], EE, rhs[:], start=True, stop=True)
                        idxw = ipool.tile([128, 8 * K], i16, tag="idxw")
                        nc.scalar.activation(idxw[:].bitcast(u16), ibc[:], ACTF.Copy)

                        flush_pending()

                        # gather c rows (k-major output: [F, K, 128])
                        gath = gpool.tile([128, K * 128], u32, tag="gath")
                        if li < 3:
                            Fg = ((F + 15) // 16) * 16
                            nc.gpsimd.ap_gather(
                                gath[0:Fg, :].bitcast(f32),
                                CTx[0:Fg, :].rearrange("c (n d) -> c n d", d=1),
                                idxw[0:Fg, :],
                                channels=Fg, num_elems=P, d=1, num_idxs=K * 128)
                        else:
                            nc.gpsimd.ap_gather(
                                gath[:],
                                CTx[:].rearrange("c (n d) -> c n d", d=1),
                                idxw[:],
                                channels=128, num_elems=P, d=1, num_idxs=K * 128)
                        pending.append((gath, tc0, tc1))

                    flush_pending()
                    if li < 3:
                        emit_prologue(li + 1, 0)
                        emit_prologue(li + 1, 1)
                    else:
                        emit_pooling_half(0)
                        emit_pooling_half(1)

                # ---------------- finish pooling ----------------
                GP = cpool.tile([128, 16], f32)
                nc.vector.tensor_tensor(GP[:, 0:8], GPH[:, 0:8], GPH[:, 8:16],
                                        op=ALU.max)
                nc.vector.tensor_tensor(GP[:, 0:8], GP[:, 0:8], MISC[:, 5:13],
                                        op=ALU.add)
                nc.vector.tensor_tensor(GP[:, 8:16], GMH[:, 0:8], GMH[:, 8:16],
                                        op=ALU.add)
                nc.vector.tensor_scalar(GP[:, 8:16], GP[:, 8:16], 1.0 / P, None,
                                        op0=ALU.mult)
                nc.vector.tensor_tensor(GP[:, 8:16], GP[:, 8:16], MISC[:, 5:13],
                                        op=ALU.add)

                # pooled [2048] -> cc_in via PE transpose (one DMA), AllGather
                gpt_ps = ibcpool.tile([128, 8 * K], f32, tag="ibc")
                nc.tensor.transpose(gpt_ps[0:16, 0:128], GP[:], IDT[:])
                GPT = wpool.tile([16, 128], f32, tag="gpt")
                nc.scalar.activation(GPT[:], gpt_ps[0:16, 0:128], ACTF.Copy)
                nc.sync.dma_start(
                    cc_in[:].rearrange("a (m f) -> m (a f)", m=16), GPT[:])
                nc.gpsimd.collective_compute(
                    "AllGather", ALU.bypass,
                    replica_groups=[list(range(N_CORES))],
                    ins=[cc_in[:].opt()], outs=[cc_out[:].opt()])

                # ---------------- head (redundant on every core) ----------------
                HTraw = gpool.tile([128, K * 128], u32, tag="gath")
                HTrawv = HTraw[0:8, 0:2048].bitcast(f32)
                nc.sync.dma_start(HTrawv, cc_out[:])
                ht_ps = ibcpool.tile([128, 8 * K], f32, tag="ibc")
                for k in range(16):
                    nc.tensor.transpose(ht_ps[:, 8 * k:8 * (k + 1)],
                                        HTrawv[:, 128 * k:128 * (k + 1)],
                                        IDT[0:8, 0:8])
                HT = cpool.tile([128, 128], f32)
                nc.scalar.activation(HT[:], ht_ps[:, 0:128], ACTF.Copy)

                def bn_leaky(src, blocks, gcol):
                    # src [128, 8*blocks]; batch-norm over batch then leaky,
                    # vectorized across blocks. gamma at MISC col gcol..,
                    # beta at gcol+blocks..
                    sv = src.rearrange("c (b e) -> c b e", b=blocks)
                    mu = wpool.tile([128, 4], f32, tag="mu")
                    nc.vector.tensor_reduce(mu[:, 0:blocks], sv, axis=AG.X,
                                            op=ALU.add)
                    nc.vector.tensor_scalar(mu[:, 0:blocks], mu[:, 0:blocks],
                                            1.0 / 8, None, op0=ALU.mult)
                    nc.vector.tensor_tensor(
                        sv, sv,
                        mu[:, 0:blocks].unsqueeze(-1).broadcast_to(
                            [128, blocks, 8]), op=ALU.subtract)
                    sq2 = wpool.tile([128, 32], f32, tag="sq2")
                    nc.scalar.activation(sq2[:, 0:8 * blocks], src, ACTF.Square)
                    var = wpool.tile([128, 4], f32, tag="var")
                    nc.vector.tensor_reduce(
                        var[:, 0:blocks],
                        sq2[:, 0:8 * blocks].rearrange("c (b e) -> c b e",
                                                       b=blocks),
                        axis=AG.X, op=ALU.add)
                    nc.scalar.activation(var[:, 0:blocks], var[:, 0:blocks],
                                         ACTF.Sqrt, scale=1.0 / 8, bias=EPSC[:])
                    nc.vector.reciprocal(var[:, 0:blocks], var[:, 0:blocks])
                    nc.vector.tensor_tensor(var[:, 0:blocks], var[:, 0:blocks],
                                            MISC[:, gcol:gcol + blocks],
                                            op=ALU.mult)
                    nc.vector.tensor_tensor(
                        sv, sv,
                        var[:, 0:blocks].unsqueeze(-1).broadcast_to(
                            [128, blocks, 8]), op=ALU.mult)
                    nc.vector.tensor_tensor(
                        sv, sv,
                        MISC[:, gcol + blocks:gcol + 2 * blocks].unsqueeze(
                            -1).broadcast_to([128, blocks, 8]), op=ALU.add)
                    lk = wpool.tile([128, 32], f32, tag="lk")
                    nc.vector.tensor_scalar(lk[:, 0:8 * blocks], src, 0.2,
                                            None, op0=ALU.mult)
                    nc.vector.tensor_tensor(src, src, lk[:, 0:8 * blocks],
                                            op=ALU.max)

                HA = cpool.tile([128, 32], f32)
                for mt in range(4):
                    hps = ibcpool.tile([128, 8 * K], f32, tag="ibc")
                    for k in range(16):
                        nc.tensor.matmul(
                            hps[:, 0:8],
                            wa_sb[:, 512 * k + 128 * mt:512 * k + 128 * (mt + 1)],
                            HT[:, 8 * k:8 * (k + 1)],
                            start=(k == 0), stop=(k == 15))
                    nc.scalar.activation(HA[:, 8 * mt:8 * (mt + 1)], hps[:, 0:8],
                                         ACTF.Identity,
                                         bias=MISC[:, 13 + mt:14 + mt])
                bn_leaky(HA[:], 4, 17)

                HB = cpool.tile([128, 16], f32)
                for mt in range(2):
                    hps = ibcpool.tile([128, 8 * K], f32, tag="ibc")
                    for k in range(4):
                        nc.tensor.matmul(
                            hps[:, 0:8],
                            wbh_sb[:, 256 * k + 128 * mt:256 * k + 128 * (mt + 1)],
                            HA[:, 8 * k:8 * (k + 1)],
                            start=(k == 0), stop=(k == 3))
                    nc.scalar.activation(HB[:, 8 * mt:8 * (mt + 1)], hps[:, 0:8],
                                         ACTF.Identity,
                                         bias=MISC[:, 25 + mt:26 + mt])
                bn_leaky(HB[:], 2, 27)

                ops2 = ibcpool.tile([128, 8 * K], f32, tag="ibc")
                for k in range(2):
                    nc.tensor.matmul(ops2[0:40, 0:8],
                                     wc_sb[:, 40 * k:40 * (k + 1)],
                                     HB[:, 8 * k:8 * (k + 1)],
                                     start=(k == 0), stop=(k == 1))
                outs = cpool.tile([40, 8], f32)
                nc.scalar.activation(outs[:], ops2[0:40, 0:8], ACTF.Identity,
                                     bias=MISC[0:40, 31:32])
                nc.sync.dma_start(y_out[:].rearrange("b f -> f b"), outs[:])

    nc.finalize()
    return nc


def _prep_inputs(inputs):
    """Host-side sharding + weight reparametrization; all fp32."""
    f = np.float32
    pos = np.ascontiguousarray(inputs["pos"], dtype=f).reshape(B, P, 3)
    names = [("W1", "b1"), ("W2", "b2"), ("W3", "b3"), ("W4", "b4")]
    common = {}
    mis = np.zeros((128, 32), dtype=f)
    for li, (C, F) in enumerate(LAYERS):
        W = np.asarray(inputs[names[li][0]], dtype=f)
        b = np.asarray(inputs[names[li][1]], dtype=f)
        common[f"wsub{li}"] = np.ascontiguousarray(W[:C] - W[C:])
        common[f"whalf{li}"] = np.ascontiguousarray(W[C:])
        if li < 3:
            mis[0:F, li] = b
        else:
            mis[:, 3] = b[0:128]
            mis[:, 4] = b[128:256]
    mis[:, 5:13] = np.asarray(inputs["bm"], dtype=f).reshape(8, 128).T
    mis[:, 13:17] = np.asarray(inputs["ba"], dtype=f).reshape(4, 128).T
    mis[:, 17:21] = np.asarray(inputs["ga"], dtype=f).reshape(4, 128).T
    mis[:, 21:25] = np.asarray(inputs["bea"], dtype=f).reshape(4, 128).T
    mis[:, 25:27] = np.asarray(inputs["bb"], dtype=f).reshape(2, 128).T
    mis[:, 27:29] = np.asarray(inputs["gb"], dtype=f).reshape(2, 128).T
    mis[:, 29:31] = np.asarray(inputs["beb"], dtype=f).reshape(2, 128).T
    mis[0:40, 31] = np.asarray(inputs["bc"], dtype=f)
    common["misc"] = mis
    # selection consts for the idx-wrap transform
    p = np.arange(128)
    i = np.arange(128)
    EE = (p[:, None] % 16 == i[None, :] % 16).astype(np.float16)
    g = np.arange(8)
    MCm = np.broadcast_to(
        (p[:, None, None] // 16 == g[None, None, :]), (128, K, 8))
    MC = MCm.astype(np.float16).reshape(128, 8 * K)
    common["cst16"] = np.ascontiguousarray(np.concatenate([EE, MC], axis=1))
    common["idt"] = np.eye(128, dtype=f)
    # wm pack [128, 5120]: k-tile i in col block 1024*i, rows 0:ki
    Wm = np.asarray(inputs["Wm"], dtype=f)
    wmp = np.zeros((128, 5120), dtype=f)
    wmp[0:64, 0:1024] = Wm[0:64]
    wmp[0:64, 1024:2048] = Wm[64:128]
    wmp[:, 2048:3072] = Wm[128:256]
    wmp[:, 3072:4096] = Wm[256:384]
    wmp[:, 4096:5120] = Wm[384:512]
    common["wm"] = wmp
    Wa = np.asarray(inputs["Wa"], dtype=f)  # [2048, 512]
    common["wa"] = np.ascontiguousarray(
        Wa.reshape(16, 128, 512).transpose(1, 0, 2).reshape(128, 8192))
    Wb = np.asarray(inputs["Wb"], dtype=f)  # [512, 256]
    common["wbh"] = np.ascontiguousarray(
        Wb.reshape(4, 128, 256).transpose(1, 0, 2).reshape(128, 1024))
    Wc = np.asarray(inputs["Wc"], dtype=f)  # [256, 40]
    common["wc"] = np.ascontiguousarray(
        Wc.reshape(2, 128, 40).transpose(1, 0, 2).reshape(128, 80))
    maps = []
    for c in range(N_CORES):
        m = dict(common)
        m["posT"] = np.ascontiguousarray(pos[c].T)
        maps.append(m)
    return maps


def kernel(**inputs) -> np.ndarray:
    from concourse.bass_utils import run_bass_kernel_spmd

    if "nc" not in _cache:
        _cache["nc"] = _build()
    nc = _cache["nc"]
    in_maps = _prep_inputs(inputs)
    res = run_bass_kernel_spmd(nc, in_maps, core_ids=list(range(N_CORES)))
    return np.asarray(res.results[0]["y"], dtype=np.float32)


# revision 28
# speedup vs baseline: 1.0395x; 1.0075x over previous
"""DGCNN point-cloud classifier on 8 Trainium2 NeuronCores.

Sharding: data-parallel over the batch dim B=8 - one point cloud per core.
Each core runs 4 dynamic-kNN edge-conv layers + the 512->1024 linear +
global max/mean pooling locally; the pooled [2048] vectors are AllGathered
and every core computes the (tiny) batch-norm MLP head redundantly.

Edge-conv algebra: h[p,k] = [x_p, x_j - x_p] @ W + b with max over k
  = (x_p @ (Wt - Wb) + b) + max_k (x_j @ Wb)
so only per-point features go through matmuls; the kNN gather moves rows of
c = x @ Wb with gpsimd ap_gather in a feature-major layout. Exact fp32
top-20 per row via DVE max8/match_replace/max_index.

The wrapped gather-index array (per gpsimd core group [16, 160], position
n = 128*k + p at (n%16, n//16), replicated into all 8 groups) is built with
ZERO DMAs: a masked broadcast-multiply followed by one tiny PE matmul
against 0/1 selection constants shipped from the host. k-major index order
makes the k-reduction a strided tensor_tensor max tree, which runs on the
otherwise-idle Pool engine. Layer 4 packs its 256 c-features as two fp16
halves in one u32 tensor so a single gather moves all of them.

Cross-layer overlap: each layer's prologue (R=2x, x^2, negsq, a/c matmuls)
is emitted per column-half as soon as the previous layer's first/second
four tiles finish, with AT/CT/NEGSQ double-buffered between layers; the
pooling matmuls similarly run in column-halves overlapped with layer 4.
"""
import numpy as np

N_CORES = 8
B, P, K, OUT = 8, 1024, 20, 40
T = P // 128  # 8 partition tiles per cloud
EPS = 1e-5
NEG = -1e30

# per-layer (C_in, F_out)
LAYERS = [(3, 64), (64, 64), (64, 128), (128, 256)]

_cache = {}


def _build():
    import concourse.bacc as bacc
    import concourse.mybir as mybir
    from concourse.tile import TileContext

    f32 = mybir.dt.float32
    f16 = mybir.dt.float16
    u16 = mybir.dt.uint16
    i16 = mybir.dt.int16
    u32 = mybir.dt.uint32
    f32r = mybir.dt.float32r

    nc = bacc.Bacc(None, num_devices=N_CORES)

    # ---------------- I/O ----------------
    posT = nc.dram_tensor("posT", [3, P], f32, kind="ExternalInput")
    wsub, whalf = [], []
    for li, (C, F) in enumerate(LAYERS):
        wsub.append(nc.dram_tensor(f"wsub{li}", [C, F], f32, kind="ExternalInput"))
        whalf.append(nc.dram_tensor(f"whalf{li}", [C, F], f32, kind="ExternalInput"))
    cst16 = nc.dram_tensor("cst16", [128, 288], f16, kind="ExternalInput")
    idt = nc.dram_tensor("idt", [128, 128], f32, kind="ExternalInput")
    misc = nc.dram_tensor("misc", [128, 32], f32, kind="ExternalInput")
    wm = nc.dram_tensor("wm", [128, 5120], f32, kind="ExternalInput")
    wa = nc.dram_tensor("wa", [128, 8192], f32, kind="ExternalInput")
    wbh = nc.dram_tensor("wbh", [128, 1024], f32, kind="ExternalInput")
    wc = nc.dram_tensor("wc", [128, 80], f32, kind="ExternalInput")
    y_out = nc.dram_tensor("y", [B, OUT], f32, kind="ExternalOutput")

    cc_in = nc.dram_tensor("cc_in", [1, 2048], f32, kind="Internal")
    cc_out = nc.dram_tensor("cc_out", [B, 2048], f32, kind="Internal",
                            addr_space="Shared")

    AG = mybir.AxisListType
    ALU = mybir.AluOpType
    ACTF = mybir.ActivationFunctionType

    with TileContext(nc) as tc:
        with tc.tile_pool(name="const", bufs=1) as cpool:
            # ---------------- resident SBUF tensors ----------------
            ONES = cpool.tile([1, P], f32)
            nc.vector.memset(ONES[:], 1.0)
            NEGCOL = cpool.tile([128, 1], f32)
            nc.vector.memset(NEGCOL[:], -0.5)
            EPSC = cpool.tile([128, 1], f32)
            nc.vector.memset(EPSC[:], EPS)

            # feature buffers (x^T per layer)
            L1 = cpool.tile([4, P], f32)
            L2 = cpool.tile([65, P], f32)
            L3 = cpool.tile([65, P], f32)
            L4 = cpool.tile([128, P], f32)
            Lbufs = [L1, L2, L3, L4]
            # double-buffered across layers (prologue overlap)
            NEGa = cpool.tile([1, P], f32)
            NEGb = cpool.tile([1, P], f32)
            AT1a = cpool.tile([128, P], f32)
            AT1b = cpool.tile([128, P], f32)
            AT2 = cpool.tile([128, P], f32)
            CT1a = cpool.tile([128, P], f32)
            CT1b = cpool.tile([128, P], f32)
            CT4 = cpool.tile([128, P], u32)   # L4 packed (f16 lo=mt0, hi=mt1)
            X4a = cpool.tile([128, P], f32)
            X4b = cpool.tile([128, P], f32)
            # cat k-tiles rounded into f32r for the pooling matmul
            catr1 = cpool.tile([64, P], f32r)
            catr2 = cpool.tile([64, P], f32r)
            catr3 = cpool.tile([128, P], f32r)
            catr4a = cpool.tile([128, P], f32r)
            catr4b = cpool.tile([128, P], f32r)

            ws_sb, wh_sb = [], []
            for li, (C, F) in enumerate(LAYERS):
                w1 = cpool.tile([C, F], f32, tag=f"ws{li}")
                w2 = cpool.tile([C, F], f32, tag=f"wh{li}")
                ws_sb.append(w1)
                wh_sb.append(w2)
            EEMC = cpool.tile([128, 288], f16)
            IDT = cpool.tile([128, 128], f32)
            MISC = cpool.tile([128, 32], f32)
            nc.sync.dma_start(L1[0:3, :], posT[:])
            nc.sync.dma_start(ws_sb[0][:], wsub[0][:])
            nc.sync.dma_start(wh_sb[0][:], whalf[0][:])
            nc.sync.dma_start(EEMC[:], cst16[:])
            nc.sync.dma_start(MISC[:], misc[:])
            nc.sync.dma_start(IDT[:], idt[:])
            nc.sync.dma_start(L1[3:4, :], ONES[:])
            nc.sync.dma_start(L2[64:65, :], ONES[:])
            nc.sync.dma_start(L3[64:65, :], ONES[:])
            EE = EEMC[:, 0:128]
            MC = EEMC[:, 128:288]
            # misc cols: 0:5 layer biases, 5:13 bm, 13:17 ba, 17:21 ga,
            # 21:25 bea, 25:27 bbh, 27:29 gb, 29:31 beb, 31 bc
            BCOL = [0, 1, 2, 3]

            wm_sb = cpool.tile([128, 5120], f32r)
            wa_sb = cpool.tile([128, 8192], f32)
            wbh_sb = cpool.tile([128, 1024], f32)
            wc_sb = cpool.tile([128, 80], f32)

            def load_big_weights():
                for li in (1, 2, 3):
                    nc.sync.dma_start(ws_sb[li][:], wsub[li][:])
                    nc.sync.dma_start(wh_sb[li][:], whalf[li][:])
                nc.sync.dma_start(wm_sb[:], wm[:].bitcast(f32r))
                nc.sync.dma_start(wbh_sb[:], wbh[:])
                nc.sync.dma_start(wc_sb[:], wc[:])
                nc.sync.dma_start(wa_sb[:], wa[:])

            with tc.tile_pool(name="ps", bufs=2, space="PSUM") as pspool, \
                 tc.tile_pool(name="ps2", bufs=2, space="PSUM") as ps2pool, \
                 tc.tile_pool(name="ibc", bufs=2, space="PSUM") as ibcpool, \
                 tc.tile_pool(name="work", bufs=2) as wpool, \
                 tc.tile_pool(name="tree", bufs=1) as tpool, \
                 tc.tile_pool(name="one", bufs=2) as opool, \
                 tc.tile_pool(name="gathp", bufs=3) as gpool, \
                 tc.tile_pool(name="idxp", bufs=3) as ipool:

                def lpars(li):
                    C, F = LAYERS[li]
                    NEGSQ = [NEGa, NEGb][li % 2]
                    if li == 3:
                        ATs, CTx = [AT1b, AT2], CT4
                    else:
                        ATs = [[AT1a, AT1b][li % 2]]
                        CTx = [CT1a, CT1b][li % 2]
                    return C, F, NEGSQ, ATs, CTx

                sqx_t = {}

                def emit_prologue(li, half):
                    C, F, NEGSQ, ATs, CTx = lpars(li)
                    Lb = Lbufs[li]
                    c0, c1 = 512 * half, 512 * (half + 1)
                    if li not in sqx_t:
                        sqx_t[li] = opool.tile([128, P], f32, tag="sqx", name=f"sqx{li}")
                    sqx = sqx_t[li]
                    with tc.high_priority(offset=-60):
                        nc.scalar.activation(sqx[0:C, c0:c1], Lb[0:C, c0:c1],
                                             ACTF.Square)
                        nps = ps2pool.tile([128, 512], f32, tag="pre")
                        nc.tensor.matmul(nps[0:1, :], NEGCOL[0:C, :],
                                         sqx[0:C, c0:c1], start=True, stop=True)
                        nc.scalar.activation(NEGSQ[0:1, c0:c1], nps[0:1, :],
                                             ACTF.Copy)
                        CT4h = CT4[:].bitcast(f16).rearrange(
                            "c (n h) -> c n h", h=2)
                        for mt in range(len(ATs)):
                            Fm = min(128, F - 128 * mt)
                            aps = ps2pool.tile([128, 512], f32, tag="pre")
                            cps = ps2pool.tile([128, 512], f32, tag="pre")
                            nc.tensor.matmul(
                                aps[0:Fm, :],
                                ws_sb[li][:, 128 * mt:128 * mt + Fm],
                                Lb[0:C, c0:c1], start=True, stop=True)
                            nc.tensor.matmul(
                                cps[0:Fm, :],
                                wh_sb[li][:, 128 * mt:128 * mt + Fm],
                                Lb[0:C, c0:c1], start=True, stop=True)
                            nc.scalar.activation(
                                ATs[mt][0:Fm, c0:c1], aps[0:Fm, :],
                                ACTF.Identity,
                                bias=MISC[0:Fm, BCOL[li] + mt:BCOL[li] + mt + 1])
                            if li == 3:
                                nc.scalar.activation(
                                    CT4h[0:Fm, c0:c1, mt], cps[0:Fm, :],
                                    ACTF.Copy)
                            else:
                                nc.scalar.activation(
                                    CTx[0:Fm, c0:c1], cps[0:Fm, :], ACTF.Copy)

                # pooling state + emitters
                GPH = cpool.tile([128, 16], f32)   # gmax: col 8*half+mt
                GMH = cpool.tile([128, 16], f32)   # gmean sums
                cat_kts = [(catr1, 0, 64), (catr2, 0, 64), (catr3, 0, 128),
                           (catr4a, 0, 128), (catr4b, 0, 128)]
                wm_kts = [(0, 64, 0), (0, 64, 1024), (0, 128, 2048),
                          (0, 128, 3072), (0, 128, 4096)]

                def emit_catr123():
                    with tc.high_priority(offset=-60):
                        nc.scalar.activation(catr1[:], L2[0:64, :], ACTF.Copy)
                        nc.scalar.activation(catr2[:], L3[0:64, :], ACTF.Copy)
                        nc.scalar.activation(catr3[:], L4[:], ACTF.Copy)

                def emit_pooling_half(half):
                    c0, c1 = 512 * half, 512 * (half + 1)
                    with tc.high_priority(offset=-60):
                        nc.scalar.activation(catr4a[:, c0:c1], X4a[:, c0:c1],
                                             ACTF.Copy)
                        nc.scalar.activation(catr4b[:, c0:c1], X4b[:, c0:c1],
                                             ACTF.Copy)
                        for mt in range(8):
                            mc0, mc1 = 128 * mt, 128 * (mt + 1)
                            ops = ps2pool.tile([128, 512], f32, tag="pre")
                            for kt, ((buf, r0, r1_), (wr0, wr1, wco)) in \
                                    enumerate(zip(cat_kts, wm_kts)):
                                nc.tensor.matmul(
                                    ops[:, :],
                                    wm_sb[wr0:wr1, wco + mc0:wco + mc1],
                                    buf[r0:r1_, c0:c1],
                                    start=(kt == 0), stop=(kt == 4))
                            gcol = 8 * half + mt
                            nc.vector.tensor_reduce(
                                GPH[:, gcol:gcol + 1], ops[:], axis=AG.X,
                                op=ALU.max)
                            osb = wpool.tile([128, P], f32, tag="scr",
                                             name="osb")
                            nc.scalar.activation(
                                osb[:, 0:512], ops[:], ACTF.Copy,
                                accum_out=GMH[:, gcol:gcol + 1])

                load_big_weights()
                emit_prologue(0, 0)
                emit_prologue(0, 1)

                for li, (C, F) in enumerate(LAYERS):
                    C, F, NEGSQ, ATs, CTx = lpars(li)
                    Lb = Lbufs[li]
                    outs_mt = [Lbufs[li + 1]] if li < 3 else [X4a, X4b]
                    lhs_kts = [(Lb, C), (ONES, 1)]
                    rhs_kts = [(Lb, C), (NEGSQ, 1)]
                    if li == 3:
                        emit_catr123()

                    pending = []

                    def flush_pending(li=li, F=F, ATs=ATs, outs_mt=outs_mt):
                        with tc.high_priority(offset=-60):
                            for (g_, tc0, tc1) in pending:
                                r1 = tpool.tile([128, 128], f32, tag="r1")
                                if li < 3:
                                    # single-instr k-max: reduce over the
                                    # outer k dim via a transposed view
                                    gv = g_[:].bitcast(f32).rearrange(
                                        "c (k p) -> c p k", k=K)
                                    nc.vector.tensor_reduce(
                                        r1[0:F, :], gv[0:F], axis=AG.X,
                                        op=ALU.max)
                                    nc.gpsimd.tensor_add(
                                        outs_mt[0][0:F, tc0:tc1], r1[0:F, :],
                                        ATs[0][0:F, tc0:tc1])
                                else:
                                    g2v = g_[:].bitcast(f16).rearrange(
                                        "c (k q) -> c k q", k=K)
                                    s10 = tpool.tile([128, 2560], f16,
                                                     tag="s10")
                                    s5 = tpool.tile([128, 1280], f16, tag="s5")
                                    s2 = tpool.tile([128, 512], f16, tag="s2")
                                    s1 = r1[:].bitcast(f16)
                                    s10v = s10[:].rearrange(
                                        "c (k q) -> c k q", k=10)
                                    s5v = s5[:].rearrange(
                                        "c (k q) -> c k q", k=5)
                                    s2v = s2[:].rearrange(
                                        "c (k q) -> c k q", k=2)
                                    nc.vector.tensor_tensor(
                                        s10v, g2v[:, 0:10], g2v[:, 10:20],
                                        op=ALU.max)
                                    nc.vector.tensor_tensor(
                                        s5v, s10v[:, 0:5], s10v[:, 5:10],
                                        op=ALU.max)
                                    nc.vector.tensor_tensor(
                                        s2v, s5v[:, 0:2], s5v[:, 2:4],
                                        op=ALU.max)
                                    nc.vector.tensor_tensor(
                                        s1, s2v[:, 0], s2v[:, 1], op=ALU.max)
                                    nc.vector.tensor_tensor(
                                        s1, s1, s5v[:, 4], op=ALU.max)
                                    s1v = s1.rearrange("c (p h) -> c p h", h=2)
                                    for mt in range(2):
                                        xf = tpool.tile([128, 128], f32,
                                                        tag=f"xf{mt}")
                                        nc.scalar.activation(
                                            xf[:], s1v[:, :, mt], ACTF.Copy)
                                        nc.gpsimd.tensor_add(
                                            outs_mt[mt][:, tc0:tc1], xf[:],
                                            ATs[mt][:, tc0:tc1])
                        pending.clear()

                    for t in range(T):
                        tc0, tc1 = 128 * t, 128 * (t + 1)
                        sps = pspool.tile([128, P], f32, tag="s")
                        for n in range(2):
                            for kt, ((lb, kk), (rb, _)) in enumerate(
                                    zip(lhs_kts, rhs_kts)):
                                nc.tensor.matmul(
                                    sps[:, 512 * n:512 * (n + 1)],
                                    lb[0:kk, tc0:tc1],
                                    rb[0:kk, 512 * n:512 * (n + 1)],
                                    start=(kt == 0),
                                    stop=(kt == len(lhs_kts) - 1))

                        # exact fp32 top-20 (values + indices) per row
                        vv = ipool.tile([128, 24], f32, tag="vv")
                        idxc = ipool.tile([128, 24], u16, tag="idxc")
                        scr = wpool.tile([128, P], f32, tag="scr")
                        nc.vector.max(vv[:, 0:8], sps[:])
                        nc.vector.max_index(idxc[:, 0:8], vv[:, 0:8], sps[:])
                        nc.vector.match_replace(scr[:], vv[:, 0:8], sps[:], NEG)
                        nc.vector.max(vv[:, 8:16], scr[:])
                        nc.vector.max_index(idxc[:, 8:16], vv[:, 8:16], scr[:])
                        nc.vector.match_replace(scr[:], vv[:, 8:16], scr[:], NEG)
                        nc.vector.max(vv[:, 16:24], scr[:])
                        nc.vector.max_index(idxc[:, 16:24], vv[:, 16:24], scr[:])

                        # wrapped idx array via maskmul + PE matmul (no DMAs):
                        # idxw[16g+r, 8k+q] = idxc[16q+r, k]  (n = 128k+p)
                        idxf = ipool.tile([128, K], f16, tag="idxf")
                        nc.scalar.activation(idxf[:], idxc[:, 0:K], ACTF.Copy)
                        rhs = ipool.tile([128, 8 * K], f16, tag="rhs")
                        nc.gpsimd.tensor_tensor(
                            rhs[:].rearrange("p (k g) -> p k g", g=8),
                            idxf[:].unsqueeze(-1).broadcast_to([128, K, 8]),
                            MC.rearrange("p (k g) -> p k g", g=8),
                            op=ALU.mult)
                        ibc = ibcpool.tile([128, 8 * K], f32, tag="ibc")
                        nc.tensor.matmul(ibc[:], EE, rhs[:], start=True, stop=True)
                        idxw = ipool.tile([128, 8 * K], i16, tag="idxw")
                        nc.scalar.activation(idxw[:].bitcast(u16), ibc[:], ACTF.Copy)

                        flush_pending()

                        # gather c rows (k-major output: [F, K, 128])
                        gath = gpool.tile([128, K * 128], u32, tag="gath")
                        if li < 3:
                            Fg = ((F + 15) // 16) * 16
                            nc.gpsimd.ap_gather(
                                gath[0:Fg, :].bitcast(f32),
                                CTx[0:Fg, :].rearrange("c (n d) -> c n d", d=1),
                                idxw[0:Fg, :],
                                channels=Fg, num_elems=P, d=1, num_idxs=K * 128)
                        else:
                            nc.gpsimd.ap_gather(
                                gath[:],
                                CTx[:].rearrange("c (n d) -> c n d", d=1),
                                idxw[:],
                                channels=128, num_elems=P, d=1, num_idxs=K * 128)
                        pending.append((gath, tc0, tc1))

                    flush_pending()
                    if li < 3:
                        emit_prologue(li + 1, 0)
                        emit_prologue(li + 1, 1)
                    else:
                        emit_pooling_half(0)
                        emit_pooling_half(1)

                # ---------------- finish pooling ----------------
                GP = cpool.tile([128, 16], f32)
                nc.vector.tensor_tensor(GP[:, 0:8], GPH[:, 0:8], GPH[:, 8:16],
                                        op=ALU.max)
                nc.vector.tensor_tensor(GP[:, 0:8], GP[:, 0:8], MISC[:, 5:13],
                                        op=ALU.add)
                nc.vector.tensor_tensor(GP[:, 8:16], GMH[:, 0:8], GMH[:, 8:16],
                                        op=ALU.add)
                nc.vector.tensor_scalar(GP[:, 8:16], GP[:, 8:16], 1.0 / P, None,
                                        op0=ALU.mult)
                nc.vector.tensor_tensor(GP[:, 8:16], GP[:, 8:16], MISC[:, 5:13],
                                        op=ALU.add)

                # pooled [2048] -> cc_in via PE transpose (one DMA), AllGather
                gpt_ps = ibcpool.tile([128, 8 * K], f32, tag="ibc")
                nc.tensor.transpose(gpt_ps[0:16, 0:128], GP[:], IDT[:])
                GPT = wpool.tile([16, 128], f32, tag="gpt")
                nc.scalar.activation(GPT[:], gpt_ps[0:16, 0:128], ACTF.Copy)
                nc.sync.dma_start(
                    cc_in[:].rearrange("a (m f) -> m (a f)", m=16), GPT[:])
                nc.gpsimd.collective_compute(
                    "AllGather", ALU.bypass,
                    replica_groups=[list(range(N_CORES))],
                    ins=[cc_in[:].opt()], outs=[cc_out[:].opt()])

                # ---------------- head (redundant on every core) ----------------
                HTraw = gpool.tile([128, K * 128], u32, tag="gath")
                HTrawv = HTraw[0:8, 0:2048].bitcast(f32)
                nc.sync.dma_start(HTrawv, cc_out[:])
                ht_ps = ibcpool.tile([128, 8 * K], f32, tag="ibc")
                for k in range(16):
                    nc.tensor.transpose(ht_ps[:, 8 * k:8 * (k + 1)],
                                        HTrawv[:, 128 * k:128 * (k + 1)],
                                        IDT[0:8, 0:8])
                HT = cpool.tile([128, 128], f32)
                nc.scalar.activation(HT[:], ht_ps[:, 0:128], ACTF.Copy)

                def bn_leaky(src, blocks, gcol):
                    # src [128, 8*blocks]; batch-norm over batch then leaky,
                    # vectorized across blocks. gamma at MISC col gcol..,
                    # beta at gcol+blocks..
                    sv = src.rearrange("c (b e) -> c b e", b=blocks)
                    mu = wpool.tile([128, 4], f32, tag="mu")
                    nc.vector.tensor_reduce(mu[:, 0:blocks], sv, axis=AG.X,
                                            op=ALU.add)
                    nc.vector.tensor_scalar(mu[:, 0:blocks], mu[:, 0:blocks],
                                            1.0 / 8, None, op0=ALU.mult)
                    nc.vector.tensor_tensor(
                        sv, sv,
                        mu[:, 0:blocks].unsqueeze(-1).broadcast_to(
                            [128, blocks, 8]), op=ALU.subtract)
                    sq2 = wpool.tile([128, 32], f32, tag="sq2")
                    nc.scalar.activation(sq2[:, 0:8 * blocks], src, ACTF.Square)
                    var = wpool.tile([128, 4], f32, tag="var")
                    nc.vector.tensor_reduce(
                        var[:, 0:blocks],
                        sq2[:, 0:8 * blocks].rearrange("c (b e) -> c b e",
                                                       b=blocks),
                        axis=AG.X, op=ALU.add)
                    nc.scalar.activation(var[:, 0:blocks], var[:, 0:blocks],
                                         ACTF.Sqrt, scale=1.0 / 8, bias=EPSC[:])
                    nc.vector.reciprocal(var[:, 0:blocks], var[:, 0:blocks])
                    nc.vector.tensor_tensor(var[:, 0:blocks], var[:, 0:blocks],
                                            MISC[:, gcol:gcol + blocks],
                                            op=ALU.mult)
                    nc.vector.tensor_tensor(
                        sv, sv,
                        var[:, 0:blocks].unsqueeze(-1).broadcast_to(
                            [128, blocks, 8]), op=ALU.mult)
                    nc.vector.tensor_tensor(
                        sv, sv,
                        MISC[:, gcol + blocks:gcol + 2 * blocks].unsqueeze(
                            -1).broadcast_to([128, blocks, 8]), op=ALU.add)
                    lk = wpool.tile([128, 32], f32, tag="lk")
                    nc.vector.tensor_scalar(lk[:, 0:8 * blocks], src, 0.2,
                                            None, op0=ALU.mult)
                    nc.vector.tensor_tensor(src, src, lk[:, 0:8 * blocks],
                                            op=ALU.max)

                HA = cpool.tile([128, 32], f32)
                for mt in range(4):
                    hps = ibcpool.tile([128, 8 * K], f32, tag="ibc")
                    for k in range(16):
                        nc.tensor.matmul(
                            hps[:, 0:8],
                            wa_sb[:, 512 * k + 128 * mt:512 * k + 128 * (mt + 1)],
                            HT[:, 8 * k:8 * (k + 1)],
                            start=(k == 0), stop=(k == 15))
                    nc.scalar.activation(HA[:, 8 * mt:8 * (mt + 1)], hps[:, 0:8],
                                         ACTF.Identity,
                                         bias=MISC[:, 13 + mt:14 + mt])
                bn_leaky(HA[:], 4, 17)

                HB = cpool.tile([128, 16], f32)
                for mt in range(2):
                    hps = ibcpool.tile([128, 8 * K], f32, tag="ibc")
                    for k in range(4):
                        nc.tensor.matmul(
                            hps[:, 0:8],
                            wbh_sb[:, 256 * k + 128 * mt:256 * k + 128 * (mt + 1)],
                            HA[:, 8 * k:8 * (k + 1)],
                            start=(k == 0), stop=(k == 3))
                    nc.scalar.activation(HB[:, 8 * mt:8 * (mt + 1)], hps[:, 0:8],
                                         ACTF.Identity,
                                         bias=MISC[:, 25 + mt:26 + mt])
                bn_leaky(HB[:], 2, 27)

                ops2 = ibcpool.tile([128, 8 * K], f32, tag="ibc")
                for k in range(2):
                    nc.tensor.matmul(ops2[0:40, 0:8],
                                     wc_sb[:, 40 * k:40 * (k + 1)],
                                     HB[:, 8 * k:8 * (k + 1)],
                                     start=(k == 0), stop=(k == 1))
                outs = cpool.tile([40, 8], f32)
                nc.scalar.activation(outs[:], ops2[0:40, 0:8], ACTF.Identity,
                                     bias=MISC[0:40, 31:32])
                nc.sync.dma_start(y_out[:].rearrange("b f -> f b"), outs[:])

    nc.finalize()
    return nc


def _prep_inputs(inputs):
    """Host-side sharding + weight reparametrization; all fp32."""
    f = np.float32
    pos = np.ascontiguousarray(inputs["pos"], dtype=f).reshape(B, P, 3)
    names = [("W1", "b1"), ("W2", "b2"), ("W3", "b3"), ("W4", "b4")]
    common = {}
    mis = np.zeros((128, 32), dtype=f)
    for li, (C, F) in enumerate(LAYERS):
        W = np.asarray(inputs[names[li][0]], dtype=f)
        b = np.asarray(inputs[names[li][1]], dtype=f)
        common[f"wsub{li}"] = np.ascontiguousarray(W[:C] - W[C:])
        common[f"whalf{li}"] = np.ascontiguousarray(W[C:])
        if li < 3:
            mis[0:F, li] = b
        else:
            mis[:, 3] = b[0:128]
            mis[:, 4] = b[128:256]
    mis[:, 5:13] = np.asarray(inputs["bm"], dtype=f).reshape(8, 128).T
    mis[:, 13:17] = np.asarray(inputs["ba"], dtype=f).reshape(4, 128).T
    mis[:, 17:21] = np.asarray(inputs["ga"], dtype=f).reshape(4, 128).T
    mis[:, 21:25] = np.asarray(inputs["bea"], dtype=f).reshape(4, 128).T
    mis[:, 25:27] = np.asarray(inputs["bb"], dtype=f).reshape(2, 128).T
    mis[:, 27:29] = np.asarray(inputs["gb"], dtype=f).reshape(2, 128).T
    mis[:, 29:31] = np.asarray(inputs["beb"], dtype=f).reshape(2, 128).T
    mis[0:40, 31] = np.asarray(inputs["bc"], dtype=f)
    common["misc"] = mis
    # selection consts for the idx-wrap transform
    p = np.arange(128)
    i = np.arange(128)
    EE = (p[:, None] % 16 == i[None, :] % 16).astype(np.float16)
    g = np.arange(8)
    MCm = np.broadcast_to(
        (p[:, None, None] // 16 == g[None, None, :]), (128, K, 8))
    MC = MCm.astype(np.float16).reshape(128, 8 * K)
    common["cst16"] = np.ascontiguousarray(np.concatenate([EE, MC], axis=1))
    common["idt"] = np.eye(128, dtype=f)
    # wm pack [128, 5120]: k-tile i in col block 1024*i, rows 0:ki
    Wm = np.asarray(inputs["Wm"], dtype=f)
    wmp = np.zeros((128, 5120), dtype=f)
    wmp[0:64, 0:1024] = Wm[0:64]
    wmp[0:64, 1024:2048] = Wm[64:128]
    wmp[:, 2048:3072] = Wm[128:256]
    wmp[:, 3072:4096] = Wm[256:384]
    wmp[:, 4096:5120] = Wm[384:512]
    common["wm"] = wmp
    Wa = np.asarray(inputs["Wa"], dtype=f)  # [2048, 512]
    common["wa"] = np.ascontiguousarray(
        Wa.reshape(16, 128, 512).transpose(1, 0, 2).reshape(128, 8192))
    Wb = np.asarray(inputs["Wb"], dtype=f)  # [512, 256]
    common["wbh"] = np.ascontiguousarray(
        Wb.reshape(4, 128, 256).transpose(1, 0, 2).reshape(128, 1024))
    Wc = np.asarray(inputs["Wc"], dtype=f)  # [256, 40]
    common["wc"] = np.ascontiguousarray(
        Wc.reshape(2, 128, 40).transpose(1, 0, 2).reshape(128, 80))
    maps = []
    for c in range(N_CORES):
        m = dict(common)
        m["posT"] = np.ascontiguousarray(pos[c].T)
        maps.append(m)
    return maps


def kernel(**inputs) -> np.ndarray:
    from concourse.bass_utils import run_bass_kernel_spmd

    if "nc" not in _cache:
        _cache["nc"] = _build()
    nc = _cache["nc"]
    in_maps = _prep_inputs(inputs)
    res = run_bass_kernel_spmd(nc, in_maps, core_ids=list(range(N_CORES)))
    return np.asarray(res.results[0]["y"], dtype=np.float32)


# revision 32
# speedup vs baseline: 1.0542x; 1.0141x over previous
"""DGCNN point-cloud classifier on 8 Trainium2 NeuronCores.

Sharding: data-parallel over the batch dim B=8 - one point cloud per core.
Each core runs 4 dynamic-kNN edge-conv layers + the 512->1024 linear +
global max/mean pooling locally; the pooled [2048] vectors are AllGathered
and every core computes the (tiny) batch-norm MLP head redundantly.

Edge-conv algebra: h[p,k] = [x_p, x_j - x_p] @ W + b with max over k
  = (x_p @ (Wt - Wb) + b) + max_k (x_j @ Wb)
so only per-point features go through matmuls; the kNN gather moves rows of
c = x @ Wb with gpsimd ap_gather in a feature-major layout. Exact fp32
top-20 per row via DVE max8/match_replace/max_index.

The wrapped gather-index array (per gpsimd core group [16, 160], position
n = 128*k + p at (n%16, n//16), replicated into all 8 groups) is built with
ZERO DMAs: a masked broadcast-multiply followed by one tiny PE matmul
against 0/1 selection constants shipped from the host. k-major index order
makes the k-reduction a strided tensor_tensor max tree, which runs on the
otherwise-idle Pool engine. Layer 4 packs its 256 c-features as two fp16
halves in one u32 tensor so a single gather moves all of them.

Cross-layer overlap: each layer's prologue (R=2x, x^2, negsq, a/c matmuls)
is emitted per column-half as soon as the previous layer's first/second
four tiles finish, with AT/CT/NEGSQ double-buffered between layers; the
pooling matmuls similarly run in column-halves overlapped with layer 4.
"""
import numpy as np

N_CORES = 8
B, P, K, OUT = 8, 1024, 20, 40
T = P // 128  # 8 partition tiles per cloud
EPS = 1e-5
NEG = -1e30

# per-layer (C_in, F_out)
LAYERS = [(3, 64), (64, 64), (64, 128), (128, 256)]

_cache = {}


def _build():
    import concourse.bacc as bacc
    import concourse.mybir as mybir
    from concourse.tile import TileContext

    f32 = mybir.dt.float32
    f16 = mybir.dt.float16
    u16 = mybir.dt.uint16
    i16 = mybir.dt.int16
    u32 = mybir.dt.uint32
    f32r = mybir.dt.float32r

    nc = bacc.Bacc(None, num_devices=N_CORES)

    # ---------------- I/O ----------------
    posT = nc.dram_tensor("posT", [3, P], f32, kind="ExternalInput")
    wsub, whalf = [], []
    for li, (C, F) in enumerate(LAYERS):
        wsub.append(nc.dram_tensor(f"wsub{li}", [C, F], f32, kind="ExternalInput"))
        whalf.append(nc.dram_tensor(f"whalf{li}", [C, F], f32, kind="ExternalInput"))
    cst16 = nc.dram_tensor("cst16", [128, 288], f16, kind="ExternalInput")
    idt = nc.dram_tensor("idt", [128, 128], f32, kind="ExternalInput")
    misc = nc.dram_tensor("misc", [128, 32], f32, kind="ExternalInput")
    wm = nc.dram_tensor("wm", [128, 5120], f32, kind="ExternalInput")
    wa = nc.dram_tensor("wa", [128, 8192], f32, kind="ExternalInput")
    wbh = nc.dram_tensor("wbh", [128, 1024], f32, kind="ExternalInput")
    wc = nc.dram_tensor("wc", [128, 80], f32, kind="ExternalInput")
    y_out = nc.dram_tensor("y", [B, OUT], f32, kind="ExternalOutput")

    cc_in = nc.dram_tensor("cc_in", [1, 2048], f32, kind="Internal")
    cc_out = nc.dram_tensor("cc_out", [B, 2048], f32, kind="Internal",
                            addr_space="Shared")

    AG = mybir.AxisListType
    ALU = mybir.AluOpType
    ACTF = mybir.ActivationFunctionType

    with TileContext(nc) as tc:
        with tc.tile_pool(name="const", bufs=1) as cpool:
            # ---------------- resident SBUF tensors ----------------
            ONES = cpool.tile([1, P], f32)
            nc.vector.memset(ONES[:], 1.0)
            NEGCOL = cpool.tile([128, 1], f32)
            nc.vector.memset(NEGCOL[:], -0.5)
            EPSC = cpool.tile([128, 1], f32)
            nc.vector.memset(EPSC[:], EPS)

            # feature buffers (x^T per layer)
            L1 = cpool.tile([4, P], f32)
            L2 = cpool.tile([65, P], f32)
            L3 = cpool.tile([65, P], f32)
            L4 = cpool.tile([128, P], f32)
            Lbufs = [L1, L2, L3, L4]
            # double-buffered across layers (prologue overlap)
            NEGa = cpool.tile([1, P], f32)
            NEGb = cpool.tile([1, P], f32)
            AT1a = cpool.tile([128, P], f32)
            AT1b = cpool.tile([128, P], f32)
            AT2 = cpool.tile([128, P], f32)
            CT1a = cpool.tile([128, P], f32)
            CT1b = cpool.tile([128, P], f32)
            CT4 = cpool.tile([128, P], u32)   # L4 packed (f16 lo=mt0, hi=mt1)
            X4a = cpool.tile([128, P], f32)
            X4b = cpool.tile([128, P], f32)
            # cat k-tiles rounded into f32r for the pooling matmul
            catr1 = cpool.tile([64, P], f32r)
            catr2 = cpool.tile([64, P], f32r)
            catr3 = cpool.tile([128, P], f32r)
            catr4a = cpool.tile([128, P], f32r)
            catr4b = cpool.tile([128, P], f32r)

            ws_sb, wh_sb = [], []
            for li, (C, F) in enumerate(LAYERS):
                w1 = cpool.tile([C, F], f32, tag=f"ws{li}")
                w2 = cpool.tile([C, F], f32, tag=f"wh{li}")
                ws_sb.append(w1)
                wh_sb.append(w2)
            EEMC = cpool.tile([128, 288], f16)
            IDT = cpool.tile([128, 128], f32)
            MISC = cpool.tile([128, 32], f32)
            # PE warm-up: one throwaway matmul starts the tensor-engine
            # p-state ramp so the first real gram runs at full speed
            with tc.tile_pool(name="warm", bufs=1, space="PSUM") as wmps:
                wps = wmps.tile([128, 512], f32)
                nc.tensor.matmul(wps[0:1, 0:64], ONES[0:1, 0:1],
                                 ONES[0:1, 0:64], start=True, stop=True)
            nc.sync.dma_start(L1[0:3, :], posT[:])
            nc.sync.dma_start(ws_sb[0][:], wsub[0][:])
            nc.sync.dma_start(wh_sb[0][:], whalf[0][:])
            nc.sync.dma_start(EEMC[:], cst16[:])
            nc.sync.dma_start(MISC[:], misc[:])
            nc.sync.dma_start(IDT[:], idt[:])
            nc.sync.dma_start(L1[3:4, :], ONES[:])
            nc.sync.dma_start(L2[64:65, :], ONES[:])
            nc.sync.dma_start(L3[64:65, :], ONES[:])
            EE = EEMC[:, 0:128]
            MC = EEMC[:, 128:288]
            # misc cols: 0:5 layer biases, 5:13 bm, 13:17 ba, 17:21 ga,
            # 21:25 bea, 25:27 bbh, 27:29 gb, 29:31 beb, 31 bc
            BCOL = [0, 1, 2, 3]

            wm_sb = cpool.tile([128, 5120], f32r)
            wa_sb = cpool.tile([128, 8192], f32)
            wbh_sb = cpool.tile([128, 1024], f32)
            wc_sb = cpool.tile([128, 80], f32)

            def load_big_weights():
                for li in (1, 2, 3):
                    nc.sync.dma_start(ws_sb[li][:], wsub[li][:])
                    nc.sync.dma_start(wh_sb[li][:], whalf[li][:])
                nc.sync.dma_start(wm_sb[:], wm[:].bitcast(f32r))
                nc.sync.dma_start(wbh_sb[:], wbh[:])
                nc.sync.dma_start(wc_sb[:], wc[:])
                nc.sync.dma_start(wa_sb[:], wa[:])

            with tc.tile_pool(name="ps", bufs=2, space="PSUM") as pspool, \
                 tc.tile_pool(name="ps2", bufs=2, space="PSUM") as ps2pool, \
                 tc.tile_pool(name="ibc", bufs=2, space="PSUM") as ibcpool, \
                 tc.tile_pool(name="work", bufs=2) as wpool, \
                 tc.tile_pool(name="tree", bufs=1) as tpool, \
                 tc.tile_pool(name="one", bufs=2) as opool, \
                 tc.tile_pool(name="gathp", bufs=3) as gpool, \
                 tc.tile_pool(name="idxp", bufs=3) as ipool:

                def lpars(li):
                    C, F = LAYERS[li]
                    NEGSQ = [NEGa, NEGb][li % 2]
                    if li == 3:
                        ATs, CTx = [AT1b, AT2], CT4
                    else:
                        ATs = [[AT1a, AT1b][li % 2]]
                        CTx = [CT1a, CT1b][li % 2]
                    return C, F, NEGSQ, ATs, CTx

                sqx_t = {}

                def emit_prologue(li, half):
                    C, F, NEGSQ, ATs, CTx = lpars(li)
                    Lb = Lbufs[li]
                    c0, c1 = 512 * half, 512 * (half + 1)
                    if li not in sqx_t:
                        sqx_t[li] = opool.tile([128, P], f32, tag="sqx", name=f"sqx{li}")
                    sqx = sqx_t[li]
                    with tc.high_priority(offset=-60):
                        nc.scalar.activation(sqx[0:C, c0:c1], Lb[0:C, c0:c1],
                                             ACTF.Square)
                        nps = ps2pool.tile([128, 512], f32, tag="pre")
                        nc.tensor.matmul(nps[0:1, :], NEGCOL[0:C, :],
                                         sqx[0:C, c0:c1], start=True, stop=True)
                        nc.scalar.activation(NEGSQ[0:1, c0:c1], nps[0:1, :],
                                             ACTF.Copy)
                        CT4h = CT4[:].bitcast(f16).rearrange(
                            "c (n h) -> c n h", h=2)
                        for mt in range(len(ATs)):
                            Fm = min(128, F - 128 * mt)
                            aps = ps2pool.tile([128, 512], f32, tag="pre")
                            cps = ps2pool.tile([128, 512], f32, tag="pre")
                            nc.tensor.matmul(
                                aps[0:Fm, :],
                                ws_sb[li][:, 128 * mt:128 * mt + Fm],
                                Lb[0:C, c0:c1], start=True, stop=True)
                            nc.tensor.matmul(
                                cps[0:Fm, :],
                                wh_sb[li][:, 128 * mt:128 * mt + Fm],
                                Lb[0:C, c0:c1], start=True, stop=True)
                            nc.scalar.activation(
                                ATs[mt][0:Fm, c0:c1], aps[0:Fm, :],
                                ACTF.Identity,
                                bias=MISC[0:Fm, BCOL[li] + mt:BCOL[li] + mt + 1])
                            if li == 3:
                                nc.scalar.activation(
                                    CT4h[0:Fm, c0:c1, mt], cps[0:Fm, :],
                                    ACTF.Copy)
                            else:
                                nc.scalar.activation(
                                    CTx[0:Fm, c0:c1], cps[0:Fm, :], ACTF.Copy)

                # pooling state + emitters
                GPH = cpool.tile([128, 16], f32)   # gmax: col 8*half+mt
                GMH = cpool.tile([128, 16], f32)   # gmean sums
                cat_kts = [(catr1, 0, 64), (catr2, 0, 64), (catr3, 0, 128),
                           (catr4a, 0, 128), (catr4b, 0, 128)]
                wm_kts = [(0, 64, 0), (0, 64, 1024), (0, 128, 2048),
                          (0, 128, 3072), (0, 128, 4096)]

                def emit_catr123():
                    with tc.high_priority(offset=-60):
                        nc.scalar.activation(catr1[:], L2[0:64, :], ACTF.Copy)
                        nc.scalar.activation(catr2[:], L3[0:64, :], ACTF.Copy)
                        nc.scalar.activation(catr3[:], L4[:], ACTF.Copy)

                def emit_pooling_half(half):
                    c0, c1 = 512 * half, 512 * (half + 1)
                    with tc.high_priority(offset=-60):
                        nc.scalar.activation(catr4a[:, c0:c1], X4a[:, c0:c1],
                                             ACTF.Copy)
                        nc.scalar.activation(catr4b[:, c0:c1], X4b[:, c0:c1],
                                             ACTF.Copy)
                        for mt in range(8):
                            mc0, mc1 = 128 * mt, 128 * (mt + 1)
                            ops = ps2pool.tile([128, 512], f32, tag="pre")
                            for kt, ((buf, r0, r1_), (wr0, wr1, wco)) in \
                                    enumerate(zip(cat_kts, wm_kts)):
                                nc.tensor.matmul(
                                    ops[:, :],
                                    wm_sb[wr0:wr1, wco + mc0:wco + mc1],
                                    buf[r0:r1_, c0:c1],
                                    start=(kt == 0), stop=(kt == 4))
                            gcol = 8 * half + mt
                            nc.vector.tensor_reduce(
                                GPH[:, gcol:gcol + 1], ops[:], axis=AG.X,
                                op=ALU.max)
                            osb = wpool.tile([128, P], f32, tag="scr",
                                             name="osb")
                            nc.scalar.activation(
                                osb[:, 0:512], ops[:], ACTF.Copy,
                                accum_out=GMH[:, gcol:gcol + 1])

                load_big_weights()
                emit_prologue(0, 0)
                emit_prologue(0, 1)

                for li, (C, F) in enumerate(LAYERS):
                    C, F, NEGSQ, ATs, CTx = lpars(li)
                    Lb = Lbufs[li]
                    outs_mt = [Lbufs[li + 1]] if li < 3 else [X4a, X4b]
                    lhs_kts = [(Lb, C), (ONES, 1)]
                    rhs_kts = [(Lb, C), (NEGSQ, 1)]
                    if li == 3:
                        emit_catr123()

                    pending = []

                    def flush_pending(li=li, F=F, ATs=ATs, outs_mt=outs_mt):
                        with tc.high_priority(offset=-60):
                            for (g_, tc0, tc1) in pending:
                                r1 = tpool.tile([128, 128], f32, tag="r1")
                                if li < 3:
                                    # single-instr k-max: reduce over the
                                    # outer k dim via a transposed view
                                    gv = g_[:].bitcast(f32).rearrange(
                                        "c (k p) -> c p k", k=K)
                                    nc.vector.tensor_reduce(
                                        r1[0:F, :], gv[0:F], axis=AG.X,
                                        op=ALU.max)
                                    nc.gpsimd.tensor_add(
                                        outs_mt[0][0:F, tc0:tc1], r1[0:F, :],
                                        ATs[0][0:F, tc0:tc1])
                                else:
                                    g2v = g_[:].bitcast(f16).rearrange(
                                        "c (k q) -> c k q", k=K)
                                    s10 = tpool.tile([128, 2560], f16,
                                                     tag="s10")
                                    s5 = tpool.tile([128, 1280], f16, tag="s5")
                                    s2 = tpool.tile([128, 512], f16, tag="s2")
                                    s1 = r1[:].bitcast(f16)
                                    s10v = s10[:].rearrange(
                                        "c (k q) -> c k q", k=10)
                                    s5v = s5[:].rearrange(
                                        "c (k q) -> c k q", k=5)
                                    s2v = s2[:].rearrange(
                                        "c (k q) -> c k q", k=2)
                                    nc.vector.tensor_tensor(
                                        s10v, g2v[:, 0:10], g2v[:, 10:20],
                                        op=ALU.max)
                                    nc.vector.tensor_tensor(
                                        s5v, s10v[:, 0:5], s10v[:, 5:10],
                                        op=ALU.max)
                                    nc.vector.tensor_tensor(
                                        s2v, s5v[:, 0:2], s5v[:, 2:4],
                                        op=ALU.max)
                                    nc.vector.tensor_tensor(
                                        s1, s2v[:, 0], s2v[:, 1], op=ALU.max)
                                    nc.vector.tensor_tensor(
                                        s1, s1, s5v[:, 4], op=ALU.max)
                                    s1v = s1.rearrange("c (p h) -> c p h", h=2)
                                    for mt in range(2):
                                        xf = tpool.tile([128, 128], f32,
                                                        tag=f"xf{mt}")
                                        nc.scalar.activation(
                                            xf[:], s1v[:, :, mt], ACTF.Copy)
                                        nc.gpsimd.tensor_add(
                                            outs_mt[mt][:, tc0:tc1], xf[:],
                                            ATs[mt][:, tc0:tc1])
                        pending.clear()

                    for t in range(T):
                        tc0, tc1 = 128 * t, 128 * (t + 1)
                        sps = pspool.tile([128, P], f32, tag="s")
                        for n in range(2):
                            for kt, ((lb, kk), (rb, _)) in enumerate(
                                    zip(lhs_kts, rhs_kts)):
                                nc.tensor.matmul(
                                    sps[:, 512 * n:512 * (n + 1)],
                                    lb[0:kk, tc0:tc1],
                                    rb[0:kk, 512 * n:512 * (n + 1)],
                                    start=(kt == 0),
                                    stop=(kt == len(lhs_kts) - 1))

                        # exact fp32 top-20 (values + indices) per row;
                        # slight per-tile priority slope keeps tile t's later
                        # rounds ahead of tile t+1's round 1 in the schedule
                        vv = ipool.tile([128, 24], f32, tag="vv")
                        idxc = ipool.tile([128, 24], u16, tag="idxc")
                        scr = wpool.tile([128, P], f32, tag="scr")
                        nc.vector.max(vv[:, 0:8], sps[:])
                        nc.vector.max_index(idxc[:, 0:8], vv[:, 0:8], sps[:])
                        nc.vector.match_replace(scr[:], vv[:, 0:8], sps[:], NEG)
                        nc.vector.max(vv[:, 8:16], scr[:])
                        nc.vector.max_index(idxc[:, 8:16], vv[:, 8:16], scr[:])
                        nc.vector.match_replace(scr[:], vv[:, 8:16], scr[:], NEG)
                        nc.vector.max(vv[:, 16:24], scr[:])
                        nc.vector.max_index(idxc[:, 16:24], vv[:, 16:24], scr[:])

                        # wrapped idx array via maskmul + PE matmul (no DMAs):
                        # idxw[16g+r, 8k+q] = idxc[16q+r, k]  (n = 128k+p)
                        idxf = ipool.tile([128, K], f16, tag="idxf")
                        nc.scalar.activation(idxf[:], idxc[:, 0:K], ACTF.Copy)
                        rhs = ipool.tile([128, 8 * K], f16, tag="rhs")
                        nc.gpsimd.tensor_tensor(
                            rhs[:].rearrange("p (k g) -> p k g", g=8),
                            idxf[:].unsqueeze(-1).broadcast_to([128, K, 8]),
                            MC.rearrange("p (k g) -> p k g", g=8),
                            op=ALU.mult)
                        ibc = ibcpool.tile([128, 8 * K], f32, tag="ibc")
                        nc.tensor.matmul(ibc[:], EE, rhs[:], start=True, stop=True)
                        idxw = ipool.tile([128, 8 * K], i16, tag="idxw")
                        nc.scalar.activation(idxw[:].bitcast(u16), ibc[:], ACTF.Copy)

                        flush_pending()

                        # gather c rows (k-major output: [F, K, 128])
                        gath = gpool.tile([128, K * 128], u32, tag="gath")
                        if li < 3:
                            Fg = ((F + 15) // 16) * 16
                            nc.gpsimd.ap_gather(
                                gath[0:Fg, :].bitcast(f32),
                                CTx[0:Fg, :].rearrange("c (n d) -> c n d", d=1),
                                idxw[0:Fg, :],
                                channels=Fg, num_elems=P, d=1, num_idxs=K * 128)
                        else:
                            nc.gpsimd.ap_gather(
                                gath[:],
                                CTx[:].rearrange("c (n d) -> c n d", d=1),
                                idxw[:],
                                channels=128, num_elems=P, d=1, num_idxs=K * 128)
                        pending.append((gath, tc0, tc1))

                    flush_pending()
                    if li < 3:
                        emit_prologue(li + 1, 0)
                        emit_prologue(li + 1, 1)
                    else:
                        emit_pooling_half(0)
                        emit_pooling_half(1)

                # ---------------- finish pooling ----------------
                GP = cpool.tile([128, 16], f32)
                nc.vector.tensor_tensor(GP[:, 0:8], GPH[:, 0:8], GPH[:, 8:16],
                                        op=ALU.max)
                nc.vector.tensor_tensor(GP[:, 0:8], GP[:, 0:8], MISC[:, 5:13],
                                        op=ALU.add)
                nc.vector.tensor_tensor(GP[:, 8:16], GMH[:, 0:8], GMH[:, 8:16],
                                        op=ALU.add)
                nc.vector.tensor_scalar(GP[:, 8:16], GP[:, 8:16], 1.0 / P, None,
                                        op0=ALU.mult)
                nc.vector.tensor_tensor(GP[:, 8:16], GP[:, 8:16], MISC[:, 5:13],
                                        op=ALU.add)

                # pooled [2048] -> cc_in via PE transpose (one DMA), AllGather
                gpt_ps = ibcpool.tile([128, 8 * K], f32, tag="ibc")
                nc.tensor.transpose(gpt_ps[0:16, 0:128], GP[:], IDT[:])
                GPT = wpool.tile([16, 128], f32, tag="gpt")
                nc.scalar.activation(GPT[:], gpt_ps[0:16, 0:128], ACTF.Copy)
                nc.sync.dma_start(
                    cc_in[:].rearrange("a (m f) -> m (a f)", m=16), GPT[:])
                nc.gpsimd.collective_compute(
                    "AllGather", ALU.bypass,
                    replica_groups=[list(range(N_CORES))],
                    ins=[cc_in[:].opt()], outs=[cc_out[:].opt()])

                # ---------------- head (redundant on every core) ----------------
                HTraw = gpool.tile([128, K * 128], u32, tag="gath")
                HTrawv = HTraw[0:8, 0:2048].bitcast(f32)
                nc.sync.dma_start(HTrawv, cc_out[:])
                ht_ps = ibcpool.tile([128, 8 * K], f32, tag="ibc")
                for k in range(16):
                    nc.tensor.transpose(ht_ps[:, 8 * k:8 * (k + 1)],
                                        HTrawv[:, 128 * k:128 * (k + 1)],
                                        IDT[0:8, 0:8])
                HT = cpool.tile([128, 128], f32)
                nc.scalar.activation(HT[:], ht_ps[:, 0:128], ACTF.Copy)

                def bn_leaky(src, blocks, gcol):
                    # src [128, 8*blocks]; batch-norm over batch then leaky,
                    # vectorized across blocks. gamma at MISC col gcol..,
                    # beta at gcol+blocks..
                    sv = src.rearrange("c (b e) -> c b e", b=blocks)
                    mu = wpool.tile([128, 4], f32, tag="mu")
                    nc.vector.tensor_reduce(mu[:, 0:blocks], sv, axis=AG.X,
                                            op=ALU.add)
                    nc.vector.tensor_scalar(mu[:, 0:blocks], mu[:, 0:blocks],
                                            1.0 / 8, None, op0=ALU.mult)
                    nc.vector.tensor_tensor(
                        sv, sv,
                        mu[:, 0:blocks].unsqueeze(-1).broadcast_to(
                            [128, blocks, 8]), op=ALU.subtract)
                    sq2 = wpool.tile([128, 32], f32, tag="sq2")
                    nc.scalar.activation(sq2[:, 0:8 * blocks], src, ACTF.Square)
                    var = wpool.tile([128, 4], f32, tag="var")
                    nc.vector.tensor_reduce(
                        var[:, 0:blocks],
                        sq2[:, 0:8 * blocks].rearrange("c (b e) -> c b e",
                                                       b=blocks),
                        axis=AG.X, op=ALU.add)
                    nc.scalar.activation(var[:, 0:blocks], var[:, 0:blocks],
                                         ACTF.Sqrt, scale=1.0 / 8, bias=EPSC[:])
                    nc.vector.reciprocal(var[:, 0:blocks], var[:, 0:blocks])
                    nc.vector.tensor_tensor(var[:, 0:blocks], var[:, 0:blocks],
                                            MISC[:, gcol:gcol + blocks],
                                            op=ALU.mult)
                    nc.vector.tensor_tensor(
                        sv, sv,
                        var[:, 0:blocks].unsqueeze(-1).broadcast_to(
                            [128, blocks, 8]), op=ALU.mult)
                    nc.vector.tensor_tensor(
                        sv, sv,
                        MISC[:, gcol + blocks:gcol + 2 * blocks].unsqueeze(
                            -1).broadcast_to([128, blocks, 8]), op=ALU.add)
                    lk = wpool.tile([128, 32], f32, tag="lk")
                    nc.vector.tensor_scalar(lk[:, 0:8 * blocks], src, 0.2,
                                            None, op0=ALU.mult)
                    nc.vector.tensor_tensor(src, src, lk[:, 0:8 * blocks],
                                            op=ALU.max)

                HA = cpool.tile([128, 32], f32)
                for mt in range(4):
                    hps = ibcpool.tile([128, 8 * K], f32, tag="ibc")
                    for k in range(16):
                        nc.tensor.matmul(
                            hps[:, 0:8],
                            wa_sb[:, 512 * k + 128 * mt:512 * k + 128 * (mt + 1)],
                            HT[:, 8 * k:8 * (k + 1)],
                            start=(k == 0), stop=(k == 15))
                    nc.scalar.activation(HA[:, 8 * mt:8 * (mt + 1)], hps[:, 0:8],
                                         ACTF.Identity,
                                         bias=MISC[:, 13 + mt:14 + mt])
                bn_leaky(HA[:], 4, 17)

                HB = cpool.tile([128, 16], f32)
                for mt in range(2):
                    hps = ibcpool.tile([128, 8 * K], f32, tag="ibc")
                    for k in range(4):
                        nc.tensor.matmul(
                            hps[:, 0:8],
                            wbh_sb[:, 256 * k + 128 * mt:256 * k + 128 * (mt + 1)],
                            HA[:, 8 * k:8 * (k + 1)],
                            start=(k == 0), stop=(k == 3))
                    nc.scalar.activation(HB[:, 8 * mt:8 * (mt + 1)], hps[:, 0:8],
                                         ACTF.Identity,
                                         bias=MISC[:, 25 + mt:26 + mt])
                bn_leaky(HB[:], 2, 27)

                ops2 = ibcpool.tile([128, 8 * K], f32, tag="ibc")
                for k in range(2):
                    nc.tensor.matmul(ops2[0:40, 0:8],
                                     wc_sb[:, 40 * k:40 * (k + 1)],
                                     HB[:, 8 * k:8 * (k + 1)],
                                     start=(k == 0), stop=(k == 1))
                outs = cpool.tile([40, 8], f32)
                nc.scalar.activation(outs[:], ops2[0:40, 0:8], ACTF.Identity,
                                     bias=MISC[0:40, 31:32])
                nc.sync.dma_start(y_out[:].rearrange("b f -> f b"), outs[:])

    nc.finalize()
    return nc


def _prep_inputs(inputs):
    """Host-side sharding + weight reparametrization; all fp32."""
    f = np.float32
    pos = np.ascontiguousarray(inputs["pos"], dtype=f).reshape(B, P, 3)
    names = [("W1", "b1"), ("W2", "b2"), ("W3", "b3"), ("W4", "b4")]
    common = {}
    mis = np.zeros((128, 32), dtype=f)
    for li, (C, F) in enumerate(LAYERS):
        W = np.asarray(inputs[names[li][0]], dtype=f)
        b = np.asarray(inputs[names[li][1]], dtype=f)
        common[f"wsub{li}"] = np.ascontiguousarray(W[:C] - W[C:])
        common[f"whalf{li}"] = np.ascontiguousarray(W[C:])
        if li < 3:
            mis[0:F, li] = b
        else:
            mis[:, 3] = b[0:128]
            mis[:, 4] = b[128:256]
    mis[:, 5:13] = np.asarray(inputs["bm"], dtype=f).reshape(8, 128).T
    mis[:, 13:17] = np.asarray(inputs["ba"], dtype=f).reshape(4, 128).T
    mis[:, 17:21] = np.asarray(inputs["ga"], dtype=f).reshape(4, 128).T
    mis[:, 21:25] = np.asarray(inputs["bea"], dtype=f).reshape(4, 128).T
    mis[:, 25:27] = np.asarray(inputs["bb"], dtype=f).reshape(2, 128).T
    mis[:, 27:29] = np.asarray(inputs["gb"], dtype=f).reshape(2, 128).T
    mis[:, 29:31] = np.asarray(inputs["beb"], dtype=f).reshape(2, 128).T
    mis[0:40, 31] = np.asarray(inputs["bc"], dtype=f)
    common["misc"] = mis
    # selection consts for the idx-wrap transform
    p = np.arange(128)
    i = np.arange(128)
    EE = (p[:, None] % 16 == i[None, :] % 16).astype(np.float16)
    g = np.arange(8)
    MCm = np.broadcast_to(
        (p[:, None, None] // 16 == g[None, None, :]), (128, K, 8))
    MC = MCm.astype(np.float16).reshape(128, 8 * K)
    common["cst16"] = np.ascontiguousarray(np.concatenate([EE, MC], axis=1))
    common["idt"] = np.eye(128, dtype=f)
    # wm pack [128, 5120]: k-tile i in col block 1024*i, rows 0:ki
    Wm = np.asarray(inputs["Wm"], dtype=f)
    wmp = np.zeros((128, 5120), dtype=f)
    wmp[0:64, 0:1024] = Wm[0:64]
    wmp[0:64, 1024:2048] = Wm[64:128]
    wmp[:, 2048:3072] = Wm[128:256]
    wmp[:, 3072:4096] = Wm[256:384]
    wmp[:, 4096:5120] = Wm[384:512]
    common["wm"] = wmp
    Wa = np.asarray(inputs["Wa"], dtype=f)  # [2048, 512]
    common["wa"] = np.ascontiguousarray(
        Wa.reshape(16, 128, 512).transpose(1, 0, 2).reshape(128, 8192))
    Wb = np.asarray(inputs["Wb"], dtype=f)  # [512, 256]
    common["wbh"] = np.ascontiguousarray(
        Wb.reshape(4, 128, 256).transpose(1, 0, 2).reshape(128, 1024))
    Wc = np.asarray(inputs["Wc"], dtype=f)  # [256, 40]
    common["wc"] = np.ascontiguousarray(
        Wc.reshape(2, 128, 40).transpose(1, 0, 2).reshape(128, 80))
    maps = []
    for c in range(N_CORES):
        m = dict(common)
        m["posT"] = np.ascontiguousarray(pos[c].T)
        maps.append(m)
    return maps


def kernel(**inputs) -> np.ndarray:
    from concourse.bass_utils import run_bass_kernel_spmd

    if "nc" not in _cache:
        _cache["nc"] = _build()
    nc = _cache["nc"]
    in_maps = _prep_inputs(inputs)
    res = run_bass_kernel_spmd(nc, in_maps, core_ids=list(range(N_CORES)))
    return np.asarray(res.results[0]["y"], dtype=np.float32)


# revision 37
# speedup vs baseline: 1.0629x; 1.0083x over previous
"""DGCNN point-cloud classifier on 8 Trainium2 NeuronCores.

Sharding: data-parallel over the batch dim B=8 - one point cloud per core.
Each core runs 4 dynamic-kNN edge-conv layers + the 512->1024 linear +
global max/mean pooling locally; the pooled [2048] vectors are AllGathered
and every core computes the (tiny) batch-norm MLP head redundantly.

Edge-conv algebra: h[p,k] = [x_p, x_j - x_p] @ W + b with max over k
  = (x_p @ (Wt - Wb) + b) + max_k (x_j @ Wb)
so only per-point features go through matmuls; the kNN gather moves rows of
c = x @ Wb with gpsimd ap_gather in a feature-major layout. Exact fp32
top-20 per row via DVE max8/match_replace/max_index.

The wrapped gather-index array (per gpsimd core group [16, 160], position
n = 128*k + p at (n%16, n//16), replicated into all 8 groups) is built with
ZERO DMAs: a masked broadcast-multiply followed by one tiny PE matmul
against 0/1 selection constants shipped from the host. k-major index order
makes the k-reduction a strided tensor_tensor max tree, which runs on the
otherwise-idle Pool engine. Layer 4 packs its 256 c-features as two fp16
halves in one u32 tensor so a single gather moves all of them.

Cross-layer overlap: each layer's prologue (R=2x, x^2, negsq, a/c matmuls)
is emitted per column-half as soon as the previous layer's first/second
four tiles finish, with AT/CT/NEGSQ double-buffered between layers; the
pooling matmuls similarly run in column-halves overlapped with layer 4.
"""
import numpy as np

N_CORES = 8
B, P, K, OUT = 8, 1024, 20, 40
T = P // 128  # 8 partition tiles per cloud
EPS = 1e-5
NEG = -1e30

# per-layer (C_in, F_out)
LAYERS = [(3, 64), (64, 64), (64, 128), (128, 256)]

_cache = {}


def _build():
    import concourse.bacc as bacc
    import concourse.mybir as mybir
    from concourse.tile import TileContext

    f32 = mybir.dt.float32
    f16 = mybir.dt.float16
    u16 = mybir.dt.uint16
    i16 = mybir.dt.int16
    u32 = mybir.dt.uint32
    f32r = mybir.dt.float32r

    nc = bacc.Bacc(None, num_devices=N_CORES)

    # ---------------- I/O ----------------
    posT = nc.dram_tensor("posT", [3, P], f32, kind="ExternalInput")
    wsub, whalf = [], []
    for li, (C, F) in enumerate(LAYERS):
        wsub.append(nc.dram_tensor(f"wsub{li}", [C, F], f32, kind="ExternalInput"))
        whalf.append(nc.dram_tensor(f"whalf{li}", [C, F], f32, kind="ExternalInput"))
    cst16 = nc.dram_tensor("cst16", [128, 288], f16, kind="ExternalInput")
    idt = nc.dram_tensor("idt", [128, 128], f32, kind="ExternalInput")
    misc = nc.dram_tensor("misc", [128, 32], f32, kind="ExternalInput")
    wm = nc.dram_tensor("wm", [128, 5120], f32, kind="ExternalInput")
    wa = nc.dram_tensor("wa", [128, 8192], f32, kind="ExternalInput")
    wbh = nc.dram_tensor("wbh", [128, 1024], f32, kind="ExternalInput")
    wc = nc.dram_tensor("wc", [128, 80], f32, kind="ExternalInput")
    y_out = nc.dram_tensor("y", [B, OUT], f32, kind="ExternalOutput")

    cc_in = nc.dram_tensor("cc_in", [1, 2048], f32, kind="Internal")
    cc_out = nc.dram_tensor("cc_out", [B, 2048], f32, kind="Internal",
                            addr_space="Shared")

    AG = mybir.AxisListType
    ALU = mybir.AluOpType
    ACTF = mybir.ActivationFunctionType

    with TileContext(nc) as tc:
        with tc.tile_pool(name="const", bufs=1) as cpool:
            # ---------------- resident SBUF tensors ----------------
            ONES = cpool.tile([1, P], f32)
            nc.vector.memset(ONES[:], 1.0)
            NEGCOL = cpool.tile([128, 1], f32)
            nc.vector.memset(NEGCOL[:], -0.5)
            EPSC = cpool.tile([128, 1], f32)
            nc.vector.memset(EPSC[:], EPS)

            # feature buffers (x^T per layer)
            L1 = cpool.tile([4, P], f32)
            L2 = cpool.tile([65, P], f32)
            L3 = cpool.tile([65, P], f32)
            L4 = cpool.tile([128, P], f32)
            Lbufs = [L1, L2, L3, L4]
            # double-buffered across layers (prologue overlap)
            NEGa = cpool.tile([1, P], f32)
            NEGb = cpool.tile([1, P], f32)
            AT1a = cpool.tile([128, P], f32)
            AT1b = cpool.tile([128, P], f32)
            AT2 = cpool.tile([128, P], f32)
            CT1a = cpool.tile([128, P], f32)
            CT1b = cpool.tile([128, P], f32)
            CT4 = cpool.tile([128, P], u32)   # L4 packed (f16 lo=mt0, hi=mt1)
            X4a = cpool.tile([128, P], f32)
            X4b = cpool.tile([128, P], f32)
            # cat k-tiles rounded into f32r for the pooling matmul
            catr1 = cpool.tile([64, P], f32r)
            catr2 = cpool.tile([64, P], f32r)
            catr3 = cpool.tile([128, P], f32r)
            catr4a = cpool.tile([128, P], f32r)
            catr4b = cpool.tile([128, P], f32r)

            ws_sb, wh_sb = [], []
            for li, (C, F) in enumerate(LAYERS):
                w1 = cpool.tile([C, F], f32, tag=f"ws{li}")
                w2 = cpool.tile([C, F], f32, tag=f"wh{li}")
                ws_sb.append(w1)
                wh_sb.append(w2)
            EEMC = cpool.tile([128, 288], f16)
            IDT = cpool.tile([128, 128], f32)
            MISC = cpool.tile([128, 32], f32)
            # PE warm-up: one throwaway matmul starts the tensor-engine
            # p-state ramp so the first real gram runs at full speed
            with tc.tile_pool(name="warm", bufs=1, space="PSUM") as wmps:
                wps = wmps.tile([128, 512], f32)
                nc.tensor.matmul(wps[0:1, 0:64], ONES[0:1, 0:1],
                                 ONES[0:1, 0:64], start=True, stop=True)
            nc.sync.dma_start(L1[0:3, :], posT[:])
            nc.sync.dma_start(ws_sb[0][:], wsub[0][:])
            nc.sync.dma_start(wh_sb[0][:], whalf[0][:])
            nc.sync.dma_start(EEMC[:], cst16[:])
            nc.sync.dma_start(MISC[:], misc[:])
            nc.sync.dma_start(IDT[:], idt[:])
            nc.sync.dma_start(L1[3:4, :], ONES[:])
            nc.sync.dma_start(L2[64:65, :], ONES[:])
            nc.sync.dma_start(L3[64:65, :], ONES[:])
            EE = EEMC[:, 0:128]
            MC = EEMC[:, 128:288]
            # misc cols: 0:5 layer biases, 5:13 bm, 13:17 ba, 17:21 ga,
            # 21:25 bea, 25:27 bbh, 27:29 gb, 29:31 beb, 31 bc
            BCOL = [0, 1, 2, 3]

            wm_sb = cpool.tile([128, 5120], f32r)
            wa_sb = cpool.tile([128, 8192], f32)
            wbh_sb = cpool.tile([128, 1024], f32)
            wc_sb = cpool.tile([128, 80], f32)

            def load_big_weights():
                for li in (1, 2, 3):
                    nc.sync.dma_start(ws_sb[li][:], wsub[li][:])
                    nc.sync.dma_start(wh_sb[li][:], whalf[li][:])
                nc.sync.dma_start(wm_sb[:], wm[:].bitcast(f32r))
                nc.sync.dma_start(wbh_sb[:], wbh[:])
                nc.sync.dma_start(wc_sb[:], wc[:])
                nc.sync.dma_start(wa_sb[:], wa[:])

            with tc.tile_pool(name="ps", bufs=2, space="PSUM") as pspool, \
                 tc.tile_pool(name="ps2", bufs=2, space="PSUM") as ps2pool, \
                 tc.tile_pool(name="ibc", bufs=2, space="PSUM") as ibcpool, \
                 tc.tile_pool(name="work", bufs=2) as wpool, \
                 tc.tile_pool(name="tree", bufs=1) as tpool, \
                 tc.tile_pool(name="one", bufs=2) as opool, \
                 tc.tile_pool(name="gathp", bufs=3) as gpool, \
                 tc.tile_pool(name="idxp", bufs=3) as ipool:

                def lpars(li):
                    C, F = LAYERS[li]
                    NEGSQ = [NEGa, NEGb][li % 2]
                    if li == 3:
                        ATs, CTx = [AT1b, AT2], CT4
                    else:
                        ATs = [[AT1a, AT1b][li % 2]]
                        CTx = [CT1a, CT1b][li % 2]
                    return C, F, NEGSQ, ATs, CTx

                sqx_t = {}

                def emit_prologue(li, half):
                    C, F, NEGSQ, ATs, CTx = lpars(li)
                    Lb = Lbufs[li]
                    c0, c1 = 512 * half, 512 * (half + 1)
                    if li not in sqx_t:
                        sqx_t[li] = opool.tile([128, P], f32, tag="sqx", name=f"sqx{li}")
                    sqx = sqx_t[li]
                    with tc.high_priority(offset=-60):
                        nc.scalar.activation(sqx[0:C, c0:c1], Lb[0:C, c0:c1],
                                             ACTF.Square)
                        nps = ps2pool.tile([128, 512], f32, tag="pre")
                        nc.tensor.matmul(nps[0:1, :], NEGCOL[0:C, :],
                                         sqx[0:C, c0:c1], start=True, stop=True)
                        nc.scalar.activation(NEGSQ[0:1, c0:c1], nps[0:1, :],
                                             ACTF.Copy)
                        CT4h = CT4[:].bitcast(f16).rearrange(
                            "c (n h) -> c n h", h=2)
                        for mt in range(len(ATs)):
                            Fm = min(128, F - 128 * mt)
                            aps = ps2pool.tile([128, 512], f32, tag="pre")
                            cps = ps2pool.tile([128, 512], f32, tag="pre")
                            nc.tensor.matmul(
                                aps[0:Fm, :],
                                ws_sb[li][:, 128 * mt:128 * mt + Fm],
                                Lb[0:C, c0:c1], start=True, stop=True)
                            nc.tensor.matmul(
                                cps[0:Fm, :],
                                wh_sb[li][:, 128 * mt:128 * mt + Fm],
                                Lb[0:C, c0:c1], start=True, stop=True)
                            nc.scalar.activation(
                                ATs[mt][0:Fm, c0:c1], aps[0:Fm, :],
                                ACTF.Identity,
                                bias=MISC[0:Fm, BCOL[li] + mt:BCOL[li] + mt + 1])
                            if li == 3:
                                nc.scalar.activation(
                                    CT4h[0:Fm, c0:c1, mt], cps[0:Fm, :],
                                    ACTF.Copy)
                            else:
                                nc.scalar.activation(
                                    CTx[0:Fm, c0:c1], cps[0:Fm, :], ACTF.Copy)

                # pooling state + emitters
                GPH = cpool.tile([128, 16], f32)   # gmax: col 8*half+mt
                GMH = cpool.tile([128, 16], f32)   # gmean sums
                cat_kts = [(catr1, 0, 64), (catr2, 0, 64), (catr3, 0, 128),
                           (catr4a, 0, 128), (catr4b, 0, 128)]
                wm_kts = [(0, 64, 0), (0, 64, 1024), (0, 128, 2048),
                          (0, 128, 3072), (0, 128, 4096)]

                CS = cpool.tile([128, 8], f32)    # cat column sums
                CSr = cpool.tile([128, 40], f32r)

                def emit_catr123():
                    with tc.high_priority(offset=-60):
                        nc.scalar.activation(catr1[:], L2[0:64, :], ACTF.Copy,
                                             accum_out=CS[0:64, 0:1])
                        nc.scalar.activation(catr2[:], L3[0:64, :], ACTF.Copy,
                                             accum_out=CS[0:64, 1:2])
                        nc.scalar.activation(catr3[:], L4[:], ACTF.Copy,
                                             accum_out=CS[:, 2:3])

                def emit_pooling_half(half):
                    c0, c1 = 512 * half, 512 * (half + 1)
                    with tc.high_priority(offset=-60):
                        nc.scalar.activation(catr4a[:, c0:c1], X4a[:, c0:c1],
                                             ACTF.Copy,
                                             accum_out=CS[:, 3 + half:4 + half])
                        nc.scalar.activation(catr4b[:, c0:c1], X4b[:, c0:c1],
                                             ACTF.Copy,
                                             accum_out=CS[:, 5 + half:6 + half])
                        for mt in range(8):
                            mc0, mc1 = 128 * mt, 128 * (mt + 1)
                            ops = ps2pool.tile([128, 512], f32, tag="pre")
                            for kt, ((buf, r0, r1_), (wr0, wr1, wco)) in \
                                    enumerate(zip(cat_kts, wm_kts)):
                                nc.tensor.matmul(
                                    ops[:, :],
                                    wm_sb[wr0:wr1, wco + mc0:wco + mc1],
                                    buf[r0:r1_, c0:c1],
                                    start=(kt == 0), stop=(kt == 4))
                            gcol = 8 * half + mt
                            nc.vector.tensor_reduce(
                                GPH[:, gcol:gcol + 1], ops[:], axis=AG.X,
                                op=ALU.max)

                load_big_weights()
                emit_prologue(0, 0)
                emit_prologue(0, 1)

                for li, (C, F) in enumerate(LAYERS):
                    C, F, NEGSQ, ATs, CTx = lpars(li)
                    Lb = Lbufs[li]
                    outs_mt = [Lbufs[li + 1]] if li < 3 else [X4a, X4b]
                    lhs_kts = [(Lb, C), (ONES, 1)]
                    rhs_kts = [(Lb, C), (NEGSQ, 1)]
                    if li == 3:
                        emit_catr123()

                    pending = []

                    def flush_pending(li=li, F=F, ATs=ATs, outs_mt=outs_mt):
                        with tc.high_priority(offset=-60):
                            for (g_, tc0, tc1) in pending:
                                r1 = tpool.tile([128, 128], f32, tag="r1")
                                if li < 3:
                                    # single-instr k-max: reduce over the
                                    # outer k dim via a transposed view
                                    gv = g_[:].bitcast(f32).rearrange(
                                        "c (k p) -> c p k", k=K)
                                    nc.vector.tensor_reduce(
                                        r1[0:F, :], gv[0:F], axis=AG.X,
                                        op=ALU.max)
                                    nc.gpsimd.tensor_add(
                                        outs_mt[0][0:F, tc0:tc1], r1[0:F, :],
                                        ATs[0][0:F, tc0:tc1])
                                else:
                                    g2v = g_[:].bitcast(f16).rearrange(
                                        "c (k q) -> c k q", k=K)
                                    s10 = tpool.tile([128, 2560], f16,
                                                     tag="s10")
                                    s5 = tpool.tile([128, 1280], f16, tag="s5")
                                    s2 = tpool.tile([128, 512], f16, tag="s2")
                                    s1 = r1[:].bitcast(f16)
                                    s10v = s10[:].rearrange(
                                        "c (k q) -> c k q", k=10)
                                    s5v = s5[:].rearrange(
                                        "c (k q) -> c k q", k=5)
                                    s2v = s2[:].rearrange(
                                        "c (k q) -> c k q", k=2)
                                    nc.vector.tensor_tensor(
                                        s10v, g2v[:, 0:10], g2v[:, 10:20],
                                        op=ALU.max)
                                    nc.vector.tensor_tensor(
                                        s5v, s10v[:, 0:5], s10v[:, 5:10],
                                        op=ALU.max)
                                    nc.vector.tensor_tensor(
                                        s2v, s5v[:, 0:2], s5v[:, 2:4],
                                        op=ALU.max)
                                    nc.vector.tensor_tensor(
                                        s1, s2v[:, 0], s2v[:, 1], op=ALU.max)
                                    nc.vector.tensor_tensor(
                                        s1, s1, s5v[:, 4], op=ALU.max)
                                    s1v = s1.rearrange("c (p h) -> c p h", h=2)
                                    for mt in range(2):
                                        xf = tpool.tile([128, 128], f32,
                                                        tag=f"xf{mt}")
                                        nc.scalar.activation(
                                            xf[:], s1v[:, :, mt], ACTF.Copy)
                                        nc.gpsimd.tensor_add(
                                            outs_mt[mt][:, tc0:tc1], xf[:],
                                            ATs[mt][:, tc0:tc1])
                        pending.clear()

                    for t in range(T):
                        tc0, tc1 = 128 * t, 128 * (t + 1)
                        sps = pspool.tile([128, P], f32, tag="s")
                        for n in range(2):
                            for kt, ((lb, kk), (rb, _)) in enumerate(
                                    zip(lhs_kts, rhs_kts)):
                                nc.tensor.matmul(
                                    sps[:, 512 * n:512 * (n + 1)],
                                    lb[0:kk, tc0:tc1],
                                    rb[0:kk, 512 * n:512 * (n + 1)],
                                    start=(kt == 0),
                                    stop=(kt == len(lhs_kts) - 1))

                        # exact fp32 top-20 (values + indices) per row;
                        # slight per-tile priority slope keeps tile t's later
                        # rounds ahead of tile t+1's round 1 in the schedule
                        vv = ipool.tile([128, 24], f32, tag="vv")
                        idxc = ipool.tile([128, 24], u16, tag="idxc")
                        scr = wpool.tile([128, P], f32, tag="scr")
                        nc.vector.max(vv[:, 0:8], sps[:])
                        nc.vector.max_index(idxc[:, 0:8], vv[:, 0:8], sps[:])
                        nc.vector.match_replace(scr[:], vv[:, 0:8], sps[:], NEG)
                        nc.vector.max(vv[:, 8:16], scr[:])
                        nc.vector.max_index(idxc[:, 8:16], vv[:, 8:16], scr[:])
                        nc.vector.match_replace(scr[:], vv[:, 8:16], scr[:], NEG)
                        nc.vector.max(vv[:, 16:24], scr[:])
                        nc.vector.max_index(idxc[:, 16:24], vv[:, 16:24], scr[:])

                        # wrapped idx array via maskmul + PE matmul (no DMAs):
                        # idxw[16g+r, 8k+q] = idxc[16q+r, k]  (n = 128k+p)
                        idxf = ipool.tile([128, K], f16, tag="idxf")
                        nc.scalar.activation(idxf[:], idxc[:, 0:K], ACTF.Copy)
                        rhs = ipool.tile([128, 8 * K], f16, tag="rhs")
                        nc.gpsimd.tensor_tensor(
                            rhs[:].rearrange("p (k g) -> p k g", g=8),
                            idxf[:].unsqueeze(-1).broadcast_to([128, K, 8]),
                            MC.rearrange("p (k g) -> p k g", g=8),
                            op=ALU.mult)
                        ibc = ibcpool.tile([128, 8 * K], f32, tag="ibc")
                        nc.tensor.matmul(ibc[:], EE, rhs[:], start=True, stop=True)
                        idxw = ipool.tile([128, 8 * K], i16, tag="idxw")
                        nc.scalar.activation(idxw[:].bitcast(u16), ibc[:], ACTF.Copy)

                        flush_pending()

                        # gather c rows (k-major output: [F, K, 128])
                        gath = gpool.tile([128, K * 128], u32, tag="gath")
                        if li < 3:
                            Fg = ((F + 15) // 16) * 16
                            nc.gpsimd.ap_gather(
                                gath[0:Fg, :].bitcast(f32),
                                CTx[0:Fg, :].rearrange("c (n d) -> c n d", d=1),
                                idxw[0:Fg, :],
                                channels=Fg, num_elems=P, d=1, num_idxs=K * 128)
                        else:
                            nc.gpsimd.ap_gather(
                                gath[:],
                                CTx[:].rearrange("c (n d) -> c n d", d=1),
                                idxw[:],
                                channels=128, num_elems=P, d=1, num_idxs=K * 128)
                        pending.append((gath, tc0, tc1))

                    flush_pending()
                    if li < 3:
                        emit_prologue(li + 1, 0)
                        emit_prologue(li + 1, 1)
                    else:
                        emit_pooling_half(0)
                        emit_pooling_half(1)

                # ---------------- finish pooling ----------------
                # gmean = (sum_p cat) @ Wm / P : tiny matvec per mt block
                nc.vector.tensor_tensor(CS[:, 3:4], CS[:, 3:4], CS[:, 4:5],
                                        op=ALU.add)
                nc.vector.tensor_tensor(CS[:, 4:5], CS[:, 5:6], CS[:, 6:7],
                                        op=ALU.add)
                cs_cols = [(0, 64, 0), (0, 64, 1), (0, 128, 2), (0, 128, 3),
                           (0, 128, 4)]
                # 8 identical rhs columns per k-tile (f32r needs wide moving)
                for kt, (csr0, csr1, csc) in enumerate(cs_cols):
                    nc.scalar.activation(
                        CSr[csr0:csr1, 8 * kt:8 * (kt + 1)].rearrange(
                            "c (a e) -> c a e", a=1),
                        CS[csr0:csr1, csc:csc + 1].unsqueeze(-1).broadcast_to(
                            [csr1 - csr0, 1, 8]), ACTF.Copy)
                gm_ps = ibcpool.tile([128, 8 * K], f32, tag="ibc")
                for mt in range(8):
                    mc0, mc1 = 128 * mt, 128 * (mt + 1)
                    for kt, ((csr0, csr1, csc), (wr0, wr1, wco)) in enumerate(
                            zip(cs_cols, wm_kts)):
                        nc.tensor.matmul(
                            gm_ps[:, 8 * mt:8 * (mt + 1)],
                            wm_sb[wr0:wr1, wco + mc0:wco + mc1],
                            CSr[csr0:csr1, 8 * kt:8 * (kt + 1)],
                            start=(kt == 0), stop=(kt == 4))
                GP = cpool.tile([128, 16], f32)
                nc.scalar.activation(
                    GP[:, 8:16],
                    gm_ps[:, 0:64].rearrange("c (m e) -> c m e", e=8)[:, :, 0],
                    ACTF.Copy)
                nc.vector.tensor_tensor(GP[:, 0:8], GPH[:, 0:8], GPH[:, 8:16],
                                        op=ALU.max)
                nc.vector.tensor_tensor(GP[:, 0:8], GP[:, 0:8], MISC[:, 5:13],
                                        op=ALU.add)
                nc.vector.tensor_scalar(GP[:, 8:16], GP[:, 8:16], 1.0 / P, None,
                                        op0=ALU.mult)
                nc.vector.tensor_tensor(GP[:, 8:16], GP[:, 8:16], MISC[:, 5:13],
                                        op=ALU.add)

                # pooled [2048] -> cc_in via PE transpose (one DMA), AllGather
                gpt_ps = ibcpool.tile([128, 8 * K], f32, tag="ibc")
                nc.tensor.transpose(gpt_ps[0:16, 0:128], GP[:], IDT[:])
                GPT = wpool.tile([16, 128], f32, tag="gpt")
                nc.scalar.activation(GPT[:], gpt_ps[0:16, 0:128], ACTF.Copy)
                nc.sync.dma_start(
                    cc_in[:].rearrange("a (m f) -> m (a f)", m=16), GPT[:])
                nc.gpsimd.collective_compute(
                    "AllGather", ALU.bypass,
                    replica_groups=[list(range(N_CORES))],
                    ins=[cc_in[:].opt()], outs=[cc_out[:].opt()])

                # ---------------- head (redundant on every core) ----------------
                HTraw = gpool.tile([128, K * 128], u32, tag="gath")
                HTrawv = HTraw[0:8, 0:2048].bitcast(f32)
                nc.sync.dma_start(HTrawv, cc_out[:])
                ht_ps = ibcpool.tile([128, 8 * K], f32, tag="ibc")
                for k in range(16):
                    nc.tensor.transpose(ht_ps[:, 8 * k:8 * (k + 1)],
                                        HTrawv[:, 128 * k:128 * (k + 1)],
                                        IDT[0:8, 0:8])
                HT = cpool.tile([128, 128], f32)
                nc.scalar.activation(HT[:], ht_ps[:, 0:128], ACTF.Copy)

                def bn_leaky(src, blocks, gcol):
                    # src [128, 8*blocks]; batch-norm over batch then leaky,
                    # vectorized across blocks. gamma at MISC col gcol..,
                    # beta at gcol+blocks..
                    sv = src.rearrange("c (b e) -> c b e", b=blocks)
                    mu = wpool.tile([128, 4], f32, tag="mu")
                    nc.vector.tensor_reduce(mu[:, 0:blocks], sv, axis=AG.X,
                                            op=ALU.add)
                    nc.vector.tensor_scalar(mu[:, 0:blocks], mu[:, 0:blocks],
                                            1.0 / 8, None, op0=ALU.mult)
                    nc.vector.tensor_tensor(
                        sv, sv,
                        mu[:, 0:blocks].unsqueeze(-1).broadcast_to(
                            [128, blocks, 8]), op=ALU.subtract)
                    sq2 = wpool.tile([128, 32], f32, tag="sq2")
                    nc.scalar.activation(sq2[:, 0:8 * blocks], src, ACTF.Square)
                    var = wpool.tile([128, 4], f32, tag="var")
                    nc.vector.tensor_reduce(
                        var[:, 0:blocks],
                        sq2[:, 0:8 * blocks].rearrange("c (b e) -> c b e",
                                                       b=blocks),
                        axis=AG.X, op=ALU.add)
                    nc.scalar.activation(var[:, 0:blocks], var[:, 0:blocks],
                                         ACTF.Sqrt, scale=1.0 / 8, bias=EPSC[:])
                    nc.vector.reciprocal(var[:, 0:blocks], var[:, 0:blocks])
                    nc.vector.tensor_tensor(var[:, 0:blocks], var[:, 0:blocks],
                                            MISC[:, gcol:gcol + blocks],
                                            op=ALU.mult)
                    nc.vector.tensor_tensor(
                        sv, sv,
                        var[:, 0:blocks].unsqueeze(-1).broadcast_to(
                            [128, blocks, 8]), op=ALU.mult)
                    nc.vector.tensor_tensor(
                        sv, sv,
                        MISC[:, gcol + blocks:gcol + 2 * blocks].unsqueeze(
                            -1).broadcast_to([128, blocks, 8]), op=ALU.add)
                    lk = wpool.tile([128, 32], f32, tag="lk")
                    nc.vector.tensor_scalar(lk[:, 0:8 * blocks], src, 0.2,
                                            None, op0=ALU.mult)
                    nc.vector.tensor_tensor(src, src, lk[:, 0:8 * blocks],
                                            op=ALU.max)

                HA = cpool.tile([128, 32], f32)
                for mt in range(4):
                    hps = ibcpool.tile([128, 8 * K], f32, tag="ibc")
                    for k in range(16):
                        nc.tensor.matmul(
                            hps[:, 0:8],
                            wa_sb[:, 512 * k + 128 * mt:512 * k + 128 * (mt + 1)],
                            HT[:, 8 * k:8 * (k + 1)],
                            start=(k == 0), stop=(k == 15))
                    nc.scalar.activation(HA[:, 8 * mt:8 * (mt + 1)], hps[:, 0:8],
                                         ACTF.Identity,
                                         bias=MISC[:, 13 + mt:14 + mt])
                bn_leaky(HA[:], 4, 17)

                HB = cpool.tile([128, 16], f32)
                for mt in range(2):
                    hps = ibcpool.tile([128, 8 * K], f32, tag="ibc")
                    for k in range(4):
                        nc.tensor.matmul(
                            hps[:, 0:8],
                            wbh_sb[:, 256 * k + 128 * mt:256 * k + 128 * (mt + 1)],
                            HA[:, 8 * k:8 * (k + 1)],
                            start=(k == 0), stop=(k == 3))
                    nc.scalar.activation(HB[:, 8 * mt:8 * (mt + 1)], hps[:, 0:8],
                                         ACTF.Identity,
                                         bias=MISC[:, 25 + mt:26 + mt])
                bn_leaky(HB[:], 2, 27)

                ops2 = ibcpool.tile([128, 8 * K], f32, tag="ibc")
                for k in range(2):
                    nc.tensor.matmul(ops2[0:40, 0:8],
                                     wc_sb[:, 40 * k:40 * (k + 1)],
                                     HB[:, 8 * k:8 * (k + 1)],
                                     start=(k == 0), stop=(k == 1))
                outs = cpool.tile([40, 8], f32)
                nc.scalar.activation(outs[:], ops2[0:40, 0:8], ACTF.Identity,
                                     bias=MISC[0:40, 31:32])
                nc.sync.dma_start(y_out[:].rearrange("b f -> f b"), outs[:])

    nc.finalize()
    return nc


def _prep_inputs(inputs):
    """Host-side sharding + weight reparametrization; all fp32."""
    f = np.float32
    pos = np.ascontiguousarray(inputs["pos"], dtype=f).reshape(B, P, 3)
    names = [("W1", "b1"), ("W2", "b2"), ("W3", "b3"), ("W4", "b4")]
    common = {}
    mis = np.zeros((128, 32), dtype=f)
    for li, (C, F) in enumerate(LAYERS):
        W = np.asarray(inputs[names[li][0]], dtype=f)
        b = np.asarray(inputs[names[li][1]], dtype=f)
        common[f"wsub{li}"] = np.ascontiguousarray(W[:C] - W[C:])
        common[f"whalf{li}"] = np.ascontiguousarray(W[C:])
        if li < 3:
            mis[0:F, li] = b
        else:
            mis[:, 3] = b[0:128]
            mis[:, 4] = b[128:256]
    mis[:, 5:13] = np.asarray(inputs["bm"], dtype=f).reshape(8, 128).T
    mis[:, 13:17] = np.asarray(inputs["ba"], dtype=f).reshape(4, 128).T
    mis[:, 17:21] = np.asarray(inputs["ga"], dtype=f).reshape(4, 128).T
    mis[:, 21:25] = np.asarray(inputs["bea"], dtype=f).reshape(4, 128).T
    mis[:, 25:27] = np.asarray(inputs["bb"], dtype=f).reshape(2, 128).T
    mis[:, 27:29] = np.asarray(inputs["gb"], dtype=f).reshape(2, 128).T
    mis[:, 29:31] = np.asarray(inputs["beb"], dtype=f).reshape(2, 128).T
    mis[0:40, 31] = np.asarray(inputs["bc"], dtype=f)
    common["misc"] = mis
    # selection consts for the idx-wrap transform
    p = np.arange(128)
    i = np.arange(128)
    EE = (p[:, None] % 16 == i[None, :] % 16).astype(np.float16)
    g = np.arange(8)
    MCm = np.broadcast_to(
        (p[:, None, None] // 16 == g[None, None, :]), (128, K, 8))
    MC = MCm.astype(np.float16).reshape(128, 8 * K)
    common["cst16"] = np.ascontiguousarray(np.concatenate([EE, MC], axis=1))
    common["idt"] = np.eye(128, dtype=f)
    # wm pack [128, 5120]: k-tile i in col block 1024*i, rows 0:ki
    Wm = np.asarray(inputs["Wm"], dtype=f)
    wmp = np.zeros((128, 5120), dtype=f)
    wmp[0:64, 0:1024] = Wm[0:64]
    wmp[0:64, 1024:2048] = Wm[64:128]
    wmp[:, 2048:3072] = Wm[128:256]
    wmp[:, 3072:4096] = Wm[256:384]
    wmp[:, 4096:5120] = Wm[384:512]
    common["wm"] = wmp
    Wa = np.asarray(inputs["Wa"], dtype=f)  # [2048, 512]
    common["wa"] = np.ascontiguousarray(
        Wa.reshape(16, 128, 512).transpose(1, 0, 2).reshape(128, 8192))
    Wb = np.asarray(inputs["Wb"], dtype=f)  # [512, 256]
    common["wbh"] = np.ascontiguousarray(
        Wb.reshape(4, 128, 256).transpose(1, 0, 2).reshape(128, 1024))
    Wc = np.asarray(inputs["Wc"], dtype=f)  # [256, 40]
    common["wc"] = np.ascontiguousarray(
        Wc.reshape(2, 128, 40).transpose(1, 0, 2).reshape(128, 80))
    maps = []
    for c in range(N_CORES):
        m = dict(common)
        m["posT"] = np.ascontiguousarray(pos[c].T)
        maps.append(m)
    return maps


def kernel(**inputs) -> np.ndarray:
    from concourse.bass_utils import run_bass_kernel_spmd

    if "nc" not in _cache:
        _cache["nc"] = _build()
    nc = _cache["nc"]
    in_maps = _prep_inputs(inputs)
    res = run_bass_kernel_spmd(nc, in_maps, core_ids=list(range(N_CORES)))
    return np.asarray(res.results[0]["y"], dtype=np.float32)
